# revision 7
# baseline (speedup 1.0000x reference)
"""Trainium2 Bass kernel for nn_DeformNet (8-core SPMD).

Strategy (N=6144 sharded across 8 cores, Ns=768 rows each):
  - The attention's K/V come from `losc = w_up (x) loc_scores + b_up`, which is
    rank-1 in the key index m.  Softmax logits per (head h, query n) are affine
    in s_m = loc_scores[m], so the full [H,N,M] attention collapses to
    E[h,n] = f(a[h,n]) with f(a) = sum_m s_m e^{a s_m} / sum_m e^{a s_m} and
    a = (kv1_h . q_h[:,n]) / sqrt(hd).  f is a smooth scalar function of a,
    evaluated on device via a Chebyshev/Clenshaw expansion whose coefficients
    are fitted on the host from the runtime loc_scores values (fp32-exact).
  - The multi-head mix + wc1 conv fold algebraically: h = wc1F @ F + wq2 @ E + cst2.
  - Per-core: G = Pxy_shard @ [feat_y | vert_y] on PE (fp32r), then the conv
    tail, BN stats via AllGather(+local sum), per-point weights P, partial
    Gram U^T diag(p) U and rhs, AllGather, replicated Jacobi solve (A is
    strongly diagonally dominant), T = U @ W + vert_x.
"""

import os
import sys
import types

import numpy as np


def _install_ntff_hook():
    """Make trace=True work under axon (antenv.axon_hooks is absent here)."""
    if "antenv.axon_hooks" in sys.modules:
        return
    try:
        import trn_agent_boot.trn_boot as tb
        hook = tb._ntff_profile_via_ctypes("/opt/axon/libaxon_pjrt.so")
        mod = types.ModuleType("antenv.axon_hooks")
        mod.get_axon_ntff_profile_hook = lambda: hook
        mod.set_axon_ntff_profile_hook = lambda h: None
        sys.modules["antenv.axon_hooks"] = mod
    except Exception:
        pass


_install_ntff_hook()

import concourse.bass as bass  # noqa: E402,F401
import concourse.bacc as bacc  # noqa: E402
import concourse.bass_isa as bass_isa  # noqa: E402
import concourse.tile as tile  # noqa: E402
import concourse.mybir as mybir  # noqa: E402
from concourse import bass_utils  # noqa: E402

f32 = mybir.dt.float32
f32r = mybir.dt.float32r
ALU = mybir.AluOpType
ACT = mybir.ActivationFunctionType
AX = mybir.AxisListType

# Problem constants
N_FULL, C, H, HD, K = 6144, 256, 4, 64, 128
NCORES = 8
LAMBDA, EPS, MIN_V, MAX_V = 10.0, 1e-5, 0.05, 0.95
AMAX, DEG = 16.0, 40
NJACOBI = 8
C2 = 2 * C

_NC_CACHE = {}


# --------------------------------------------------------------------------
# Device program
# --------------------------------------------------------------------------

def build(n_full=N_FULL, ncores=NCORES, use_f32r=True):
    """Build the SPMD program (identical on all cores; data differs)."""
    ns = n_full // ncores          # rows per core
    nch = ns // 128                # n-chunks (128 rows each)
    mch = n_full // 128            # m-chunks over the contraction dim
    nb = ns // 128                 # 128-wide blocks of the local free dim
    blk = 384 if ns % 384 == 0 else (256 if ns % 256 == 0 else 128)
    nblk = ns // blk               # wide free blocks for matmul streaming
    bpb = blk // 128               # 128-blocks per wide block
    mmdt = f32r if use_f32r else f32
    no2 = C2 // 128                # h channel chunks (4)
    nf = C // 128                  # feature channel chunks (2)

    nc = bacc.Bacc("TRN2", target_bir_lowering=False, debug=False,
                   num_devices=ncores, enable_asserts=False)

    def din(name, shape):
        return nc.dram_tensor(name, shape, f32, kind="ExternalInput").ap()

    # per-core inputs
    pxyT = din("pxyT", [n_full, ns])
    fxs = din("fxs", [ns, C])
    vxs = din("vxs", [ns, 3])
    us = din("us", [ns, K])
    # replicated inputs
    fv = din("fv", [n_full, C + 4])  # feat_y | vert_y | zero pad
    evl = din("evl", [K, 1])           # LAMBDA * evals
    waT = din("waT", [128, nf * H])    # (kv1_h^T wq_h)/sqrt(hd), chunked
    bav = din("bav", [H, 1])           # (kv1_h . bq_h)/sqrt(hd)
    chb = din("chb", [128, DEG + 1])   # Chebyshev coeffs, bcast over partitions
    wc1FT = din("wc1FT", [128, nf * C2])  # wc1[:, :C].T chunked
    wq2T = din("wq2T", [H, C2])        # (wc1[:, C:] @ wv1h).T
    cst2 = din("cst2", [128, no2])
    bn1g = din("bn1g", [128, no2])
    bn1b = din("bn1b", [128, no2])
    wc2T = din("wc2T", [128, no2 * C])
    cst3 = din("cst3", [128, nf])
    fwg = din("fwg", [128, nf])
    fwbb = din("fwbb", [128, nf])
    fwwT = din("fwwT", [128, nf])
    fwb0 = din("fwb0", [1, 1])
    ident = din("ident", [128, 128])

    t_out = nc.dram_tensor("t_out", [ns, 3], f32, kind="ExternalOutput").ap()
    p_out = nc.dram_tensor("p_out", [ns], f32, kind="ExternalOutput").ap()

    rg = [list(range(ncores))]
    inv_n = 1.0 / float(n_full)
    inv_nc2 = 1.0 / (float(n_full) * float(C))
    nst1 = 2 * no2 + 2
    nst2 = 2 * nf

    with tile.TileContext(nc) as tc:
        with tc.tile_pool(name="const", bufs=1) as cpool, \
             tc.tile_pool(name="fvp", bufs=1) as fvpool, \
             tc.tile_pool(name="persist", bufs=1) as pers, \
             tc.tile_pool(name="work", bufs=3) as work, \
             tc.tile_pool(name="small", bufs=2) as small, \
             tc.tile_pool(name="pxy", bufs=3) as pxyp, \
             tc.tile_pool(name="dram", bufs=1, space="DRAM") as dram:

            # ---- constants to SBUF ----
            def cdma(name, src, shape, dt=f32):
                t = cpool.tile(shape, dt, name=name)
                nc.sync.dma_start(t[:], src[:].bitcast(dt))
                return t

            ident_sb = cdma("ident_sb", ident, [128, 128])
            chb_sb = cdma("chb_sb", chb, [128, DEG + 1])
            waT_sb = cdma("waT_sb", waT, [128, nf * H], dt=mmdt)
            bav_sb = cdma("bav_sb", bav, [H, 1])
            evl_sb = cdma("evl_sb", evl, [K, 1])
            wc1FT_sb = cdma("wc1FT_sb", wc1FT, [128, nf * C2], dt=mmdt)
            wq2T_sb = cdma("wq2T_sb", wq2T, [H, C2], dt=mmdt)
            cst2_sb = cdma("cst2_sb", cst2, [128, no2])
            bn1g_sb = cdma("bn1g_sb", bn1g, [128, no2])
            bn1b_sb = cdma("bn1b_sb", bn1b, [128, no2])
            wc2T_sb = cdma("wc2T_sb", wc2T, [128, no2 * C], dt=mmdt)
            cst3_sb = cdma("cst3_sb", cst3, [128, nf])
            fwg_sb = cdma("fwg_sb", fwg, [128, nf])
            fwbb_sb = cdma("fwbb_sb", fwbb, [128, nf])
            fwwT_sb = cdma("fwwT_sb", fwwT, [128, nf], dt=mmdt)
            fwb0_sb = cdma("fwb0_sb", fwb0, [1, 1])

            # ---- shard-local tensors ----
            fxs_t = [pers.tile([128, C], f32, name=f"fxs{i}") for i in range(nch)]
            vxs_t = [pers.tile([128, 3], f32, name=f"vxs{i}") for i in range(nch)]
            uvf = [pers.tile([128, K + 3], f32, name=f"uvf{i}") for i in range(nch)]
            for i in range(nch):
                nc.sync.dma_start(fxs_t[i][:], fxs[i * 128:(i + 1) * 128, :])
                nc.sync.dma_start(vxs_t[i][:], vxs[i * 128:(i + 1) * 128, :])
                nc.sync.dma_start(uvf[i][:, 0:K], us[i * 128:(i + 1) * 128, :])

            # fv tiles (feat_y | vert_y), m-chunked, resident
            fv_t = [fvpool.tile([128, C + 4], mmdt, name=f"fv{m}")
                    for m in range(mch)]
            for m in range(mch):
                nc.sync.dma_start(fv_t[m][:],
                                  fv[m * 128:(m + 1) * 128, :].bitcast(mmdt))

            # ================= Phase 1: G = Pxy_sh @ [feat_y|vert_y] ========
            F_sb = [pers.tile([128, C], f32, name=f"F{i}") for i in range(nch)]
            sg6 = small.tile([128, nch], f32, name="sg6", bufs=1)
            with tc.tile_pool(name="gps", bufs=1, space="PSUM") as gpsp:
                gps = [gpsp.tile([128, C + 4], f32, name=f"gps{i}")
                       for i in range(nch)]
                for m in range(mch):
                    pxt = pxyp.tile([128, ns], mmdt, name="pxt")
                    nc.sync.dma_start(
                        pxt[:], pxyT[m * 128:(m + 1) * 128, :].bitcast(mmdt))
                    for i in range(nch):
                        nc.tensor.matmul(
                            gps[i][:],
                            pxt[:, i * 128:(i + 1) * 128],
                            fv_t[m][:],
                            start=(m == 0), stop=(m == mch - 1),
                        )

                # ====== Phase 2: F = G - fx ; vF = G - vx ; sigma2 ==========
                sq_scr = work.tile([128, C], f32, name="sq_scr", tag="scr")
                for i in range(nch):
                    nc.vector.tensor_sub(F_sb[i][:], gps[i][:, 0:C], fxs_t[i][:])
                    nc.vector.tensor_sub(uvf[i][:, K:K + 3],
                                         gps[i][:, C:C + 3], vxs_t[i][:])
                    nc.scalar.activation(sq_scr[:], F_sb[i][:], ACT.Square,
                                         accum_out=sg6[:, i:i + 1])

            # ============== Phase 3-5: FT, a, Clenshaw ======================
            FT = [pers.tile([128, ns], mmdt, name=f"FT{c}") for c in range(nf)]
            ef = small.tile([128, nb * H], f32, name="ef", bufs=1)
            e_sb = small.tile([H, ns], mmdt, name="e_sb", bufs=1)
            with tc.tile_pool(name="psT1", bufs=1, space="PSUM") as psT1:
                for i in range(nch):
                    for c in range(nf):
                        tp = psT1.tile([128, 128], f32, name="tp", tag="tp",
                                       bufs=2)
                        nc.tensor.transpose(tp[:],
                                            F_sb[i][:, c * 128:(c + 1) * 128],
                                            ident_sb[:])
                        nc.scalar.copy(FT[c][:, i * 128:(i + 1) * 128], tp[:])

                # a = waT^T @ FT + ba   [H, ns]
                a_sb = small.tile([H, ns], f32, name="a_sb", bufs=1)
                for b in range(nblk):
                    sl = slice(b * blk, (b + 1) * blk)
                    aps = psT1.tile([H, blk], f32, name="aps", tag="aps", bufs=2)
                    for c in range(nf):
                        nc.tensor.matmul(
                            aps[:],
                            waT_sb[:, c * H:(c + 1) * H],
                            FT[c][:, sl],
                            start=(c == 0), stop=(c == nf - 1))
                    nc.scalar.activation(a_sb[:, sl], aps[:], ACT.Identity,
                                         bias=bav_sb[:, 0:1])

                # transpose a[H, ns] -> af24 [128, nb*H] (col q = b*H + h)
                af24 = psT1.tile([128, nb * H], f32, name="af24", tag="af24",
                                 bufs=1)
                for b in range(nb):
                    nc.tensor.transpose(af24[:, b * H:(b + 1) * H],
                                        a_sb[:, b * 128:(b + 1) * 128],
                                        ident_sb[0:H, 0:H])
                # x = clip(a / AMAX, -1, 1);  t2 = 2x
                x_sb = small.tile([128, nb * H], f32, name="x_sb", bufs=1)
                nc.scalar.activation(x_sb[:], af24[:], ACT.Copy,
                                     scale=1.0 / AMAX)
                nc.vector.tensor_scalar_min(x_sb[:], x_sb[:], 1.0)
                nc.vector.tensor_scalar_max(x_sb[:], x_sb[:], -1.0)
                t2_sb = small.tile([128, nb * H], f32, name="t2_sb", bufs=1)
                nc.vector.tensor_scalar_mul(t2_sb[:], x_sb[:], 2.0)

                # Clenshaw recurrence for E = f(a)
                with tc.tile_pool(name="clen", bufs=4) as clp:
                    bprev = clp.tile([128, nb * H], f32, name="clp0", tag="cl")
                    nc.vector.memset(bprev[:], 0.0)
                    bcur = clp.tile([128, nb * H], f32, name="clc0", tag="cl")
                    nc.vector.memset(bcur[:], 0.0)
                    for kk in range(DEG, 0, -1):
                        tmp = clp.tile([128, nb * H], f32, name="clt", tag="cl")
                        nc.vector.tensor_mul(tmp[:], t2_sb[:], bcur[:])
                        bnew = clp.tile([128, nb * H], f32, name="cln", tag="cl")
                        nc.vector.scalar_tensor_tensor(
                            bnew[:], tmp[:], chb_sb[:, kk:kk + 1], bprev[:],
                            op0=ALU.add, op1=ALU.subtract)
                        bprev, bcur = bcur, bnew
                    # E_f = c0 + x*bcur - bprev
                    nc.vector.tensor_mul(ef[:], x_sb[:], bcur[:])
                    nc.vector.scalar_tensor_tensor(
                        ef[:], ef[:], chb_sb[:, 0:1], bprev[:],
                        op0=ALU.add, op1=ALU.subtract)

                # transpose back per block: E [H, ns]
                e4ps = psT1.tile([H, ns], f32, name="e4ps", tag="etp", bufs=1)
                for b in range(nb):
                    nc.tensor.transpose(e4ps[:, b * 128:(b + 1) * 128],
                                        ef[:, b * H:(b + 1) * H], ident_sb[:])
                nc.scalar.copy(e_sb[:], e4ps[:])

            # ============== Phase 6: h = wc1F@F + wq2@E + cst2; stats =======
            h_sb = [pers.tile([128, ns], f32, name=f"h{o}") for o in range(no2)]
            st1 = small.tile([128, nst1], f32, name="st1", bufs=1)
            scr = work.tile([128, blk], f32, name="hscr", tag="scr2")
            with tc.tile_pool(name="hps", bufs=1, space="PSUM") as hpsp:
                for o in range(no2):
                    osl = slice(o * 128, (o + 1) * 128)
                    acc = [small.tile([128, 1], f32, name=f"hacc{o}_{b}",
                                      tag="hacc") for b in range(nblk)]
                    sqa = [small.tile([128, 1], f32, name=f"hsq{o}_{b}",
                                      tag="hsq") for b in range(nblk)]
                    for b in range(nblk):
                        sl = slice(b * blk, (b + 1) * blk)
                        hp = hpsp.tile([128, blk], f32, name="hp", tag="hp",
                                       bufs=4)
                        for c in range(nf):
                            nc.tensor.matmul(
                                hp[:],
                                wc1FT_sb[:, c * C2 + o * 128:
                                         c * C2 + (o + 1) * 128],
                                FT[c][:, sl],
                                start=(c == 0), stop=False)
                        nc.tensor.matmul(
                            hp[:], wq2T_sb[:, osl], e_sb[:, sl],
                            start=False, stop=True)
                        # copy to SBUF with +cst2 bias, accumulate sum(h)
                        nc.scalar.activation(
                            h_sb[o][:, sl], hp[:], ACT.Identity,
                            bias=cst2_sb[:, o:o + 1], accum_out=acc[b][:])
                        nc.scalar.activation(scr[:], h_sb[o][:, sl],
                                             ACT.Square, accum_out=sqa[b][:])
                    if nblk == 1:
                        nc.vector.tensor_copy(st1[:, o:o + 1], acc[0][:])
                        nc.vector.tensor_copy(st1[:, no2 + o:no2 + o + 1],
                                              sqa[0][:])
                    else:
                        nc.vector.tensor_add(st1[:, o:o + 1], acc[0][:],
                                             acc[1][:])
                        nc.vector.tensor_add(st1[:, no2 + o:no2 + o + 1],
                                             sqa[0][:], sqa[1][:])

            # sigma2 partial into st1 col 2*nо2; zero pad col
            nc.vector.reduce_sum(st1[:, 2 * no2:2 * no2 + 1], sg6[:], axis=AX.X)
            nc.vector.memset(st1[:, 2 * no2 + 1:2 * no2 + 2], 0.0)

            # ============== AG1: bn1 stats + sigma2 =========================
            b1i = dram.tile([128, nst1], f32, name="b1i")
            b1o = dram.tile([128 * ncores, nst1], f32, addr_space="Shared",
                            name="b1o")
            nc.sync.dma_start(b1i[:], st1[:])
            nc.gpsimd.collective_compute(
                "AllGather", ALU.bypass, replica_groups=rg,
                ins=[b1i.opt()], outs=[b1o.opt()])
            st1g = small.tile([128, ncores * nst1], f32, name="st1g", bufs=1)
            nc.sync.dma_start(
                st1g.rearrange("p (r f) -> p r f", r=ncores),
                b1o.rearrange("(r p) f -> p r f", p=128))
            st1s = small.tile([128, nst1], f32, name="st1s", bufs=1)
            nc.vector.reduce_sum(
                st1s[:], st1g.rearrange("p (r f) -> p f r", r=ncores), axis=AX.X)

            # alpha/beta for bn1
            mh1 = small.tile([128, no2], f32, name="mh1", bufs=1)
            nc.vector.tensor_scalar_mul(mh1[:], st1s[:, 0:no2], inv_n)
            vh1 = small.tile([128, no2], f32, name="vh1", bufs=1)
            nc.vector.tensor_scalar_mul(vh1[:], st1s[:, no2:2 * no2], inv_n)
            msq = small.tile([128, no2], f32, name="msq", bufs=1)
            nc.vector.tensor_mul(msq[:], mh1[:], mh1[:])
            nc.vector.tensor_sub(vh1[:], vh1[:], msq[:])
            nc.vector.tensor_scalar_add(vh1[:], vh1[:], EPS)
            nc.scalar.sqrt(vh1[:], vh1[:])
            rsd = small.tile([128, no2], f32, name="rsd", bufs=1)
            nc.vector.reciprocal(rsd[:], vh1[:])
            al1 = small.tile([128, no2], f32, name="al1", bufs=1)
            nc.vector.tensor_mul(al1[:], bn1g_sb[:], rsd[:])
            be1 = small.tile([128, no2], f32, name="be1", bufs=1)
            nc.vector.tensor_mul(be1[:], al1[:], mh1[:])
            nc.vector.tensor_sub(be1[:], bn1b_sb[:], be1[:])

            # sigma2 total -> lse = LAMBDA*evals*sigma2
            sgp = small.tile([128, 1], f32, name="sgp", bufs=1)
            nc.gpsimd.partition_all_reduce(
                sgp[:], st1s[:, 2 * no2:2 * no2 + 1], channels=128,
                reduce_op=bass_isa.ReduceOp.add)
            lse = small.tile([128, 1], f32, name="lse", bufs=1)
            nc.vector.tensor_mul(lse[:], evl_sb[:], sgp[:])
            nc.vector.tensor_scalar_mul(lse[:], lse[:], inv_nc2)

            # ============== Phase 7: relu(bn1(h)); fp = F + wc2@. + cst3 ====
            rh = [pers.tile([128, ns], mmdt, name=f"rh{o}") for o in range(no2)]
            for o in range(no2):
                nc.scalar.activation(rh[o][:], h_sb[o][:], ACT.Relu,
                                     bias=be1[:, o:o + 1], scale=al1[:, o:o + 1])

            fp_sb = [pers.tile([128, ns], f32, name=f"fp{c}") for c in range(nf)]
            st2 = small.tile([128, nst2], f32, name="st2", bufs=1)
            with tc.tile_pool(name="fps", bufs=1, space="PSUM") as fpsp:
                for o in range(nf):
                    osl = slice(o * 128, (o + 1) * 128)
                    acc = [small.tile([128, 1], f32, name=f"facc{o}_{b}",
                                      tag="facc") for b in range(nblk)]
                    sqa = [small.tile([128, 1], f32, name=f"fsq{o}_{b}",
                                      tag="fsq") for b in range(nblk)]
                    for b in range(nblk):
                        sl = slice(b * blk, (b + 1) * blk)
                        fpp = fpsp.tile([128, blk], f32, name="fpp", tag="fpp",
                                        bufs=4)
                        for c in range(no2):
                            nc.tensor.matmul(
                                fpp[:],
                                wc2T_sb[:, c * C + o * 128:
                                        c * C + (o + 1) * 128],
                                rh[c][:, sl],
                                start=(c == 0), stop=(c == no2 - 1))
                        nc.vector.scalar_tensor_tensor(
                            fp_sb[o][:, sl], fpp[:], cst3_sb[:, o:o + 1],
                            FT[o][:, sl].bitcast(f32), op0=ALU.add, op1=ALU.add,
                            accum_out=acc[b][:])
                        nc.scalar.activation(scr[:], fp_sb[o][:, sl],
                                             ACT.Square, accum_out=sqa[b][:])
                    if nblk == 1:
                        nc.vector.tensor_copy(st2[:, o:o + 1], acc[0][:])
                        nc.vector.tensor_copy(st2[:, nf + o:nf + o + 1],
                                              sqa[0][:])
                    else:
                        nc.vector.tensor_add(st2[:, o:o + 1], acc[0][:],
                                             acc[1][:])
                        nc.vector.tensor_add(st2[:, nf + o:nf + o + 1],
                                             sqa[0][:], sqa[1][:])

            # ============== AG2: fw_bn stats ================================
            b2i = dram.tile([128, nst2], f32, name="b2i")
            b2o = dram.tile([128 * ncores, nst2], f32, addr_space="Shared",
                            name="b2o")
            nc.sync.dma_start(b2i[:], st2[:])
            nc.gpsimd.collective_compute(
                "AllGather", ALU.bypass, replica_groups=rg,
                ins=[b2i.opt()], outs=[b2o.opt()])
            st2g = small.tile([128, ncores * nst2], f32, name="st2g", bufs=1)
            nc.sync.dma_start(
                st2g.rearrange("p (r f) -> p r f", r=ncores),
                b2o.rearrange("(r p) f -> p r f", p=128))
            st2s = small.tile([128, nst2], f32, name="st2s", bufs=1)
            nc.vector.reduce_sum(
                st2s[:], st2g.rearrange("p (r f) -> p f r", r=ncores), axis=AX.X)

            mh2 = small.tile([128, nf], f32, name="mh2", bufs=1)
            nc.vector.tensor_scalar_mul(mh2[:], st2s[:, 0:nf], inv_n)
            vh2 = small.tile([128, nf], f32, name="vh2", bufs=1)
            nc.vector.tensor_scalar_mul(vh2[:], st2s[:, nf:2 * nf], inv_n)
            msq2 = small.tile([128, nf], f32, name="msq2", bufs=1)
            nc.vector.tensor_mul(msq2[:], mh2[:], mh2[:])
            nc.vector.tensor_sub(vh2[:], vh2[:], msq2[:])
            nc.vector.tensor_scalar_add(vh2[:], vh2[:], EPS)
            nc.scalar.sqrt(vh2[:], vh2[:])
            rsd2 = small.tile([128, nf], f32, name="rsd2", bufs=1)
            nc.vector.reciprocal(rsd2[:], vh2[:])
            al2 = small.tile([128, nf], f32, name="al2", bufs=1)
            nc.vector.tensor_mul(al2[:], fwg_sb[:], rsd2[:])
            be2 = small.tile([128, nf], f32, name="be2", bufs=1)
            nc.vector.tensor_mul(be2[:], al2[:], mh2[:])
            nc.vector.tensor_sub(be2[:], fwbb_sb[:], be2[:])

            # ============== Phase 8: P = clip(sigmoid(fw conv)) =============
            with tc.tile_pool(name="psT2", bufs=1, space="PSUM") as psT2:
                rfp = [work.tile([128, ns], mmdt, name=f"rfp{c}", tag="rfp")
                       for c in range(nf)]
                for c in range(nf):
                    nc.scalar.activation(rfp[c][:], fp_sb[c][:], ACT.Relu,
                                         bias=be2[:, c:c + 1],
                                         scale=al2[:, c:c + 1])
                p_sb = small.tile([1, ns], f32, name="p_sb", bufs=1)
                for b in range(nblk):
                    sl = slice(b * blk, (b + 1) * blk)
                    zps = psT2.tile([1, blk], f32, name="zps", tag="zps", bufs=2)
                    for c in range(nf):
                        nc.tensor.matmul(
                            zps[:],
                            fwwT_sb[:, c:c + 1],
                            rfp[c][:, sl],
                            start=(c == 0), stop=(c == nf - 1))
                    nc.scalar.activation(p_sb[:, sl], zps[:], ACT.Sigmoid,
                                         bias=fwb0_sb[:, 0:1])
                nc.vector.tensor_scalar_min(p_sb[:], p_sb[:], MAX_V)
                nc.vector.tensor_scalar_max(p_sb[:], p_sb[:], MIN_V)
                nc.sync.dma_start(p_out.rearrange("(o n) -> o n", o=1), p_sb[:])

                # transpose p -> [128, nch]
                ptp = psT2.tile([128, nch], f32, name="ptp", tag="ptp", bufs=1)
                for i in range(nch):
                    nc.tensor.transpose(ptp[:, i:i + 1],
                                        p_sb[:, i * 128:(i + 1) * 128],
                                        ident_sb[0:1, 0:1])
                pt_sb = small.tile([128, nch], f32, name="pt_sb", bufs=1)
                nc.scalar.copy(pt_sb[:], ptp[:])

                # ========== Phase 9: Gram partial + AG3 =====================
                gram_sb = small.tile([128, K + 3], f32, name="gram_sb", bufs=1)
                with tc.tile_pool(name="grp", bufs=1, space="PSUM") as grp:
                    gram = grp.tile([128, K + 3], f32, name="gram")
                    for i in range(nch):
                        pu = work.tile([128, K], f32, name="pu", tag="pu")
                        nc.vector.tensor_scalar_mul(pu[:], uvf[i][:, 0:K],
                                                    pt_sb[:, i:i + 1])
                        nc.tensor.matmul(gram[:], pu[:], uvf[i][:],
                                         start=(i == 0), stop=(i == nch - 1))
                    nc.vector.tensor_copy(gram_sb[:], gram[:])

                b3i = dram.tile([128, K + 3], f32, name="b3i")
                b3o = dram.tile([128 * ncores, K + 3], f32, addr_space="Shared",
                                name="b3o")
                nc.sync.dma_start(b3i[:], gram_sb[:])
                nc.gpsimd.collective_compute(
                    "AllGather", ALU.bypass, replica_groups=rg,
                    ins=[b3i.opt()], outs=[b3o.opt()])
                g3g = small.tile([128, ncores * (K + 3)], f32, name="g3g",
                                 bufs=1)
                nc.sync.dma_start(
                    g3g.rearrange("p (r f) -> p r f", r=ncores),
                    b3o.rearrange("(r p) f -> p r f", p=128))
                g3s = small.tile([128, K + 3], f32, name="g3s", bufs=1)
                nc.vector.reduce_sum(
                    g3s[:], g3g.rearrange("p (r f) -> p f r", r=ncores),
                    axis=AX.X)

                # A = gram_sum + diag(lse)
                A_sb = small.tile([128, K], f32, name="A_sb", bufs=1)
                nc.vector.scalar_tensor_tensor(
                    A_sb[:], ident_sb[:], lse[:, 0:1], g3s[:, 0:K],
                    op0=ALU.mult, op1=ALU.add)

                # ========== Phase 10: Jacobi solve ==========================
                dg = small.tile([128, K], f32, name="dg", bufs=1)
                dvec = small.tile([128, 1], f32, name="dvec", bufs=1)
                nc.vector.tensor_mul(dg[:], A_sb[:], ident_sb[:])
                nc.vector.reduce_sum(dvec[:], dg[:], axis=AX.X)
                dinv = small.tile([128, 1], f32, name="dinv", bufs=1)
                nc.vector.reciprocal(dinv[:], dvec[:])
                with tc.tile_pool(name="jac", bufs=4) as jac:
                    x = jac.tile([128, 3], f32, name="x0", tag="x")
                    nc.vector.tensor_scalar_mul(x[:], g3s[:, K:K + 3],
                                                dinv[:, 0:1])
                    for it in range(NJACOBI):
                        axp = psT2.tile([128, 3], f32, name="axp", tag="axp",
                                        bufs=2)
                        nc.tensor.matmul(axp[:], A_sb[:], x[:],
                                         start=True, stop=True)
                        r = jac.tile([128, 3], f32, name="r", tag="x")
                        nc.vector.tensor_sub(r[:], g3s[:, K:K + 3], axp[:])
                        xn = jac.tile([128, 3], f32, name="xn", tag="x")
                        nc.vector.scalar_tensor_tensor(
                            xn[:], r[:], dinv[:, 0:1], x[:],
                            op0=ALU.mult, op1=ALU.add)
                        x = xn

                    # ====== Phase 11: T = U @ W + vert_x ====================
                    for i in range(nch):
                        utp = psT2.tile([128, 128], f32, name="utp", tag="utp",
                                        bufs=2)
                        nc.tensor.transpose(utp[:], uvf[i][:, 0:K], ident_sb[:])
                        ut_sb = work.tile([128, 128], f32, name="ut_sb",
                                          tag="ut")
                        nc.scalar.copy(ut_sb[:], utp[:])
                        tps = psT2.tile([128, 3], f32, name="tps", tag="axp",
                                        bufs=2)
                        nc.tensor.matmul(tps[:], ut_sb[:], x[:],
                                         start=True, stop=True)
                        t_sb = work.tile([128, 3], f32, name="t_sb", tag="tsb")
                        nc.vector.tensor_add(t_sb[:], tps[:], vxs_t[i][:])
                        nc.sync.dma_start(t_out[i * 128:(i + 1) * 128, :],
                                          t_sb[:])

    nc.compile()
    return nc


# --------------------------------------------------------------------------
# Host-side prep
# --------------------------------------------------------------------------

def host_prep(inputs, n_full=N_FULL, ncores=NCORES, deg=DEG, amax=AMAX):
    """Build per-core in_maps from full inputs (layout prep + tiny algebra)."""
    ns = n_full // ncores
    fp = np.float32
    g = {k: np.asarray(v, dtype=fp) for k, v in inputs.items()}

    s = g["loc_scores"].astype(np.float64)
    # Chebyshev fit of f(a) = sum(s e^{as})/sum(e^{as}) on [-amax, amax]
    M = 4 * (deg + 1)
    nodes = np.cos(np.pi * (np.arange(M) + 0.5) / M) * amax
    t = np.exp(nodes[:, None] * s[None, :])
    fe = (t * s).sum(1) / t.sum(1)
    ch = np.polynomial.chebyshev.Chebyshev.fit(nodes, fe, deg,
                                               domain=[-amax, amax])
    cc = ch.coef.astype(fp)                     # [deg+1]
    chb = np.repeat(cc[None, :], 128, axis=0)   # [128, deg+1]

    wk, wv, wq = g["wk"], g["wv"], g["wq"]
    w_up, b_up = g["w_up"][:, 0], g["b_up"]
    kv1 = wk @ w_up                              # [C]
    vv1 = wv @ w_up
    vv0 = wv @ b_up + g["bv"]
    sq = np.float64(np.sqrt(HD))
    waT = np.zeros((C, H), fp)
    bav = np.zeros((H, 1), fp)
    for h in range(H):
        sl = slice(h * HD, (h + 1) * HD)
        waT[:, h] = (kv1[sl] @ wq[sl, :]) / sq
        bav[h, 0] = (kv1[sl] @ g["bq"][sl]) / sq

    def chunk_rows(mat):
        # [R, X] -> [128, (R//128)*X]: col block k holds rows k*128..
        R, X = mat.shape
        return np.ascontiguousarray(
            mat.reshape(R // 128, 128, X).transpose(1, 0, 2).reshape(
                128, (R // 128) * X)).astype(fp)

    wmh, bmh = g["wmh"], g["bmh"]
    wv1 = wmh * vv1[None, :]                     # [C, C]
    wv1h = np.stack([wv1[:, h * HD:(h + 1) * HD].sum(1)
                     for h in range(H)], axis=1)  # [C, H]
    const_add = wmh @ vv0 + bmh                  # [C]
    wc1, bc1 = g["wc1"], g["bc1"]
    wc1F, wc1A = wc1[:, :C], wc1[:, C:]
    wq2 = wc1A @ wv1h                            # [C2, H]
    cst2v = wc1A @ const_add + bc1               # [C2]

    def chunk_cols(v):
        # [M] -> [128, M//128]  (channel c = col*128 + p)
        return np.ascontiguousarray(v.reshape(-1, 128).T).astype(fp)

    rep = {
        "fv": np.ascontiguousarray(np.concatenate(
            [g["feat_y"][0], g["vert_y"][0],
             np.zeros((n_full, 1), fp)], axis=1)),
        "evl": (LAMBDA * g["evals_x"]).reshape(K, 1).astype(fp),
        "waT": chunk_rows(waT), "bav": bav, "chb": chb.astype(fp),
        "wc1FT": chunk_rows(np.ascontiguousarray(wc1F.T)),
        "wq2T": np.ascontiguousarray(wq2.T),
        "cst2": chunk_cols(cst2v),
        "bn1g": chunk_cols(g["bn1_g"]), "bn1b": chunk_cols(g["bn1_b"]),
        "wc2T": chunk_rows(np.ascontiguousarray(g["wc2"].T)),
        "cst3": chunk_cols(g["bc2"]),
        "fwg": chunk_cols(g["fw_bn_g"]), "fwbb": chunk_cols(g["fw_bn_b"]),
        "fwwT": chunk_rows(np.ascontiguousarray(g["fw_w"].T)),
        "fwb0": g["fw_b"].reshape(1, 1).astype(fp),
        "ident": np.eye(128, dtype=fp),
    }
    in_maps = []
    for c in range(ncores):
        sl = slice(c * ns, (c + 1) * ns)
        m = dict(rep)
        m["pxyT"] = np.ascontiguousarray(g["Pxy"][sl, :].T)
        m["fxs"] = np.ascontiguousarray(g["feat_x"][0][sl, :])
        m["vxs"] = np.ascontiguousarray(g["vert_x"][0][sl, :])
        m["us"] = np.ascontiguousarray(g["evecs_x"][sl, :])
        in_maps.append(m)
    return in_maps


def assemble(results, n_full=N_FULL, ncores=NCORES):
    ns = n_full // ncores
    T = np.zeros((1, n_full, 3), np.float32)
    P = np.zeros((1, 1, n_full), np.float32)
    for c in range(ncores):
        sl = slice(c * ns, (c + 1) * ns)
        T[0, sl, :] = results[c]["t_out"]
        P[0, 0, sl] = results[c]["p_out"]
    return T, P


def kernel(**inputs):
    key = "main"
    if key not in _NC_CACHE:
        _NC_CACHE[key] = build()
    nc = _NC_CACHE[key]
    in_maps = host_prep(inputs)
    res = bass_utils.run_bass_kernel_spmd(
        nc, in_maps, core_ids=list(range(NCORES)),
        trace=bool(os.environ.get("KERNEL_TRACE")))
    out = assemble(res.results)
    kernel.last_result = res
    return out


# revision 13
# speedup vs baseline: 1.2035x; 1.2035x over previous
"""Trainium2 Bass kernel for nn_DeformNet (8-core SPMD).

Strategy (N=6144 sharded across 8 cores, Ns=768 rows each):
  - The attention's K/V come from `losc = w_up (x) loc_scores + b_up`, which is
    rank-1 in the key index m.  Softmax logits per (head h, query n) are affine
    in s_m = loc_scores[m], so the full [H,N,M] attention collapses to
    E[h,n] = f(a[h,n]) with f(a) = sum_m s_m e^{a s_m} / sum_m e^{a s_m} and
    a = (kv1_h . q_h[:,n]) / sqrt(hd).  f is a smooth scalar function of a,
    evaluated on device via a Chebyshev/Clenshaw expansion whose coefficients
    are fitted on the host from the runtime loc_scores values (fp32-exact).
  - The multi-head mix + wc1 conv fold algebraically: h = wc1F @ F + wq2 @ E + cst2.
  - Per-core: G = Pxy_shard @ [feat_y | vert_y] on PE (fp32r), then the conv
    tail, BN stats via AllGather(+local sum), per-point weights P, partial
    Gram U^T diag(p) U and rhs, AllGather, replicated Jacobi solve (A is
    strongly diagonally dominant), T = U @ W + vert_x.
"""

import os
import sys
import types

import numpy as np


def _install_ntff_hook():
    """Make trace=True work under axon (antenv.axon_hooks is absent here)."""
    if "antenv.axon_hooks" in sys.modules:
        return
    try:
        import trn_agent_boot.trn_boot as tb
        hook = tb._ntff_profile_via_ctypes("/opt/axon/libaxon_pjrt.so")
        mod = types.ModuleType("antenv.axon_hooks")
        mod.get_axon_ntff_profile_hook = lambda: hook
        mod.set_axon_ntff_profile_hook = lambda h: None
        sys.modules["antenv.axon_hooks"] = mod
    except Exception:
        pass


_install_ntff_hook()

import concourse.bass as bass  # noqa: E402,F401
import concourse.bacc as bacc  # noqa: E402
import concourse.bass_isa as bass_isa  # noqa: E402
import concourse.tile as tile  # noqa: E402
import concourse.mybir as mybir  # noqa: E402
from concourse import bass_utils  # noqa: E402

f32 = mybir.dt.float32
f32r = mybir.dt.float32r
ALU = mybir.AluOpType
ACT = mybir.ActivationFunctionType
AX = mybir.AxisListType

# Problem constants
N_FULL, C, H, HD, K = 6144, 256, 4, 64, 128
NCORES = 8
LAMBDA, EPS, MIN_V, MAX_V = 10.0, 1e-5, 0.05, 0.95
AMAX, DEG = 16.0, 40
NJACOBI = 8
C2 = 2 * C

_NC_CACHE = {}


# --------------------------------------------------------------------------
# Device program
# --------------------------------------------------------------------------

def build(n_full=N_FULL, ncores=NCORES, use_f32r=True):
    """Build the SPMD program (identical on all cores; data differs)."""
    ns = n_full // ncores          # rows per core
    nch = ns // 128                # n-chunks (128 rows each)
    mch = n_full // 128            # m-chunks over the contraction dim
    nb = ns // 128                 # 128-wide blocks of the local free dim
    blk = 384 if ns % 384 == 0 else (256 if ns % 256 == 0 else 128)
    nblk = ns // blk               # wide free blocks for matmul streaming
    bpb = blk // 128               # 128-blocks per wide block
    mmdt = f32r if use_f32r else f32
    no2 = C2 // 128                # h channel chunks (4)
    nf = C // 128                  # feature channel chunks (2)

    nc = bacc.Bacc("TRN2", target_bir_lowering=False, debug=False,
                   num_devices=ncores, enable_asserts=False)

    def din(name, shape):
        return nc.dram_tensor(name, shape, f32, kind="ExternalInput").ap()

    gsz = mch // 8                 # fv m-chunks per DMA group
    # per-core inputs (partition-major where it kills DMA descriptors)
    pxyT = din("pxyT", [n_full, ns])
    fxsr = din("fxsr", [128, nch * C])
    vxsr = din("vxsr", [128, nch * 3])
    usr = din("usr", [128, nch * K])
    # replicated inputs
    fvr = din("fvr", [128, mch * (C + 4)])  # feat_y|vert_y|pad, partition-major
    evl = din("evl", [K, 1])           # LAMBDA * evals
    waT = din("waT", [128, nf * H])    # (kv1_h^T wq_h)/sqrt(hd), chunked
    bav = din("bav", [H, 1])           # (kv1_h . bq_h)/sqrt(hd)
    chb = din("chb", [128, DEG + 1])   # Chebyshev coeffs, bcast over partitions
    wc1FT = din("wc1FT", [128, nf * C2])  # wc1[:, :C].T chunked
    wq2T = din("wq2T", [H, C2])        # (wc1[:, C:] @ wv1h).T
    cst2 = din("cst2", [128, no2])
    bn1g = din("bn1g", [128, no2])
    bn1b = din("bn1b", [128, no2])
    wc2T = din("wc2T", [128, no2 * C])
    cst3 = din("cst3", [128, nf])
    fwg = din("fwg", [128, nf])
    fwbb = din("fwbb", [128, nf])
    fwwT = din("fwwT", [128, nf])
    fwb0 = din("fwb0", [1, 1])
    ident = din("ident", [128, 128])

    t_out = nc.dram_tensor("t_out", [3, ns], f32, kind="ExternalOutput").ap()
    p_out = nc.dram_tensor("p_out", [ns], f32, kind="ExternalOutput").ap()

    rg = [list(range(ncores))]
    inv_n = 1.0 / float(n_full)
    inv_nc2 = 1.0 / (float(n_full) * float(C))
    nst1 = 2 * no2 + 2
    nst2 = 2 * nf

    with tile.TileContext(nc) as tc:
        with tc.tile_pool(name="const", bufs=1) as cpool, \
             tc.tile_pool(name="fvp", bufs=1) as fvpool, \
             tc.tile_pool(name="persist", bufs=1) as pers, \
             tc.tile_pool(name="work", bufs=3) as work, \
             tc.tile_pool(name="small", bufs=2) as small, \
             tc.tile_pool(name="pxy", bufs=10) as pxyp, \
             tc.tile_pool(name="dram", bufs=1, space="DRAM") as dram:

            # ---- constants to SBUF ----
            def cdma(name, src, shape, dt=f32):
                t = cpool.tile(shape, dt, name=name)
                nc.sync.dma_start(t[:], src[:].bitcast(dt))
                return t

            ident_sb = cdma("ident_sb", ident, [128, 128])
            chb_sb = cdma("chb_sb", chb, [128, DEG + 1])
            waT_sb = cdma("waT_sb", waT, [128, nf * H], dt=mmdt)
            bav_sb = cdma("bav_sb", bav, [H, 1])
            evl_sb = cdma("evl_sb", evl, [K, 1])
            wc1FT_sb = cdma("wc1FT_sb", wc1FT, [128, nf * C2], dt=mmdt)
            wq2T_sb = cdma("wq2T_sb", wq2T, [H, C2], dt=mmdt)
            cst2_sb = cdma("cst2_sb", cst2, [128, no2])
            bn1g_sb = cdma("bn1g_sb", bn1g, [128, no2])
            bn1b_sb = cdma("bn1b_sb", bn1b, [128, no2])
            wc2T_sb = cdma("wc2T_sb", wc2T, [128, no2 * C], dt=mmdt)
            cst3_sb = cdma("cst3_sb", cst3, [128, nf])
            fwg_sb = cdma("fwg_sb", fwg, [128, nf])
            fwbb_sb = cdma("fwbb_sb", fwbb, [128, nf])
            fwwT_sb = cdma("fwwT_sb", fwwT, [128, nf], dt=mmdt)
            fwb0_sb = cdma("fwb0_sb", fwb0, [1, 1])

            # ---- shard-local tensors (merged DMAs) ----
            fxsbig = pers.tile([128, nch * C], f32, name="fxsbig")
            nc.sync.dma_start(fxsbig[:], fxsr[:])
            vxsbig = pers.tile([128, nch * 3], f32, name="vxsbig")
            nc.sync.dma_start(vxsbig[:], vxsr[:])
            usbig = pers.tile([128, nch * K], f32, name="usbig")
            nc.sync.dma_start(usbig[:], usr[:])
            vfb = pers.tile([128, nch * 3], f32, name="vfb")

            # fv (feat_y | vert_y | pad): 8 grouped DMAs, m-chunk slices
            cw = C + 4
            fvg = [fvpool.tile([128, gsz * cw], mmdt, name=f"fvg{g}")
                   for g in range(8)]

            def fv_t(m):
                return fvg[m // gsz][:, (m % gsz) * cw:(m % gsz + 1) * cw]

            # ================= Phase 1: G = Pxy_sh @ [feat_y|vert_y] ========
            F_sb = [pers.tile([128, C], f32, name=f"F{i}") for i in range(nch)]
            sg6 = small.tile([128, nch], f32, name="sg6", bufs=1)
            with tc.tile_pool(name="gps", bufs=1, space="PSUM") as gpsp:
                gps = [gpsp.tile([128, C + 4], f32, name=f"gps{i}")
                       for i in range(nch)]
                for m in range(mch):
                    if m % gsz == 0:
                        g = m // gsz
                        nc.sync.dma_start(
                            fvg[g][:],
                            fvr[:, g * gsz * cw:(g + 1) * gsz * cw].bitcast(mmdt))
                    pxt = pxyp.tile([128, ns], mmdt, name="pxt")
                    nc.sync.dma_start(
                        pxt[:], pxyT[m * 128:(m + 1) * 128, :].bitcast(mmdt))
                    for i in range(nch):
                        nc.tensor.matmul(
                            gps[i][:],
                            pxt[:, i * 128:(i + 1) * 128],
                            fv_t(m),
                            start=(m == 0), stop=(m == mch - 1),
                        )

                # ====== Phase 2: F = G - fx ; vF = G - vx ; sigma2 ==========
                sq_scr = work.tile([128, C], f32, name="sq_scr", tag="scr")
                for i in range(nch):
                    nc.vector.tensor_sub(F_sb[i][:], gps[i][:, 0:C],
                                         fxsbig[:, i * C:(i + 1) * C])
                    nc.vector.tensor_sub(vfb[:, i * 3:(i + 1) * 3],
                                         gps[i][:, C:C + 3],
                                         vxsbig[:, i * 3:(i + 1) * 3])
                    nc.scalar.activation(sq_scr[:], F_sb[i][:], ACT.Square,
                                         accum_out=sg6[:, i:i + 1])

            # ============== Phase 3-5: FT, a, Clenshaw ======================
            FT = [pers.tile([128, ns], mmdt, name=f"FT{c}") for c in range(nf)]
            ef = small.tile([128, nb * H], f32, name="ef", bufs=1)
            e_sb = small.tile([H, ns], mmdt, name="e_sb", bufs=1)
            with tc.tile_pool(name="psT1", bufs=1, space="PSUM") as psT1:
                for i in range(nch):
                    for c in range(nf):
                        tp = psT1.tile([128, 128], f32, name="tp", tag="tp",
                                       bufs=2)
                        nc.tensor.transpose(tp[:],
                                            F_sb[i][:, c * 128:(c + 1) * 128],
                                            ident_sb[:])
                        nc.scalar.copy(FT[c][:, i * 128:(i + 1) * 128], tp[:])

                # a = waT^T @ FT + ba   [H, ns]
                a_sb = small.tile([H, ns], f32, name="a_sb", bufs=1)
                for b in range(nblk):
                    sl = slice(b * blk, (b + 1) * blk)
                    aps = psT1.tile([H, blk], f32, name="aps", tag="aps", bufs=2)
                    for c in range(nf):
                        nc.tensor.matmul(
                            aps[:],
                            waT_sb[:, c * H:(c + 1) * H],
                            FT[c][:, sl],
                            start=(c == 0), stop=(c == nf - 1))
                    nc.scalar.activation(a_sb[:, sl], aps[:], ACT.Identity,
                                         bias=bav_sb[:, 0:1])

                # transpose a[H, ns] -> af24 [128, nb*H] (col q = b*H + h)
                af24 = psT1.tile([128, nb * H], f32, name="af24", tag="af24",
                                 bufs=1)
                for b in range(nb):
                    nc.tensor.transpose(af24[:, b * H:(b + 1) * H],
                                        a_sb[:, b * 128:(b + 1) * 128],
                                        ident_sb[0:H, 0:H])
                # x = clip(a / AMAX, -1, 1);  t2 = 2x
                x_sb = small.tile([128, nb * H], f32, name="x_sb", bufs=1)
                nc.scalar.activation(x_sb[:], af24[:], ACT.Copy,
                                     scale=1.0 / AMAX)
                nc.vector.tensor_scalar_min(x_sb[:], x_sb[:], 1.0)
                nc.vector.tensor_scalar_max(x_sb[:], x_sb[:], -1.0)
                t2_sb = small.tile([128, nb * H], f32, name="t2_sb", bufs=1)
                nc.vector.tensor_scalar_mul(t2_sb[:], x_sb[:], 2.0)

                # Clenshaw recurrence for E = f(a)
                with tc.tile_pool(name="clen", bufs=4) as clp:
                    bprev = clp.tile([128, nb * H], f32, name="clp0", tag="cl")
                    nc.vector.memset(bprev[:], 0.0)
                    bcur = clp.tile([128, nb * H], f32, name="clc0", tag="cl")
                    nc.vector.memset(bcur[:], 0.0)
                    for kk in range(DEG, 0, -1):
                        tmp = clp.tile([128, nb * H], f32, name="clt", tag="cl")
                        nc.vector.tensor_mul(tmp[:], t2_sb[:], bcur[:])
                        bnew = clp.tile([128, nb * H], f32, name="cln", tag="cl")
                        nc.vector.scalar_tensor_tensor(
                            bnew[:], tmp[:], chb_sb[:, kk:kk + 1], bprev[:],
                            op0=ALU.add, op1=ALU.subtract)
                        bprev, bcur = bcur, bnew
                    # E_f = c0 + x*bcur - bprev
                    nc.vector.tensor_mul(ef[:], x_sb[:], bcur[:])
                    nc.vector.scalar_tensor_tensor(
                        ef[:], ef[:], chb_sb[:, 0:1], bprev[:],
                        op0=ALU.add, op1=ALU.subtract)

                # transpose back per block: E [H, ns]
                e4ps = psT1.tile([H, ns], f32, name="e4ps", tag="etp", bufs=1)
                for b in range(nb):
                    nc.tensor.transpose(e4ps[:, b * 128:(b + 1) * 128],
                                        ef[:, b * H:(b + 1) * H], ident_sb[:])
                nc.scalar.copy(e_sb[:], e4ps[:])

            # ============== Phase 6: h = wc1F@F + wq2@E + cst2; stats =======
            h_sb = [pers.tile([128, ns], f32, name=f"h{o}") for o in range(no2)]
            st1 = small.tile([128, nst1], f32, name="st1", bufs=1)
            scr = work.tile([128, blk], f32, name="hscr", tag="scr2")
            with tc.tile_pool(name="hps", bufs=1, space="PSUM") as hpsp:
                for o in range(no2):
                    osl = slice(o * 128, (o + 1) * 128)
                    acc = [small.tile([128, 1], f32, name=f"hacc{o}_{b}",
                                      tag="hacc") for b in range(nblk)]
                    sqa = [small.tile([128, 1], f32, name=f"hsq{o}_{b}",
                                      tag="hsq") for b in range(nblk)]
                    for b in range(nblk):
                        sl = slice(b * blk, (b + 1) * blk)
                        hp = hpsp.tile([128, blk], f32, name="hp", tag="hp",
                                       bufs=4)
                        for c in range(nf):
                            nc.tensor.matmul(
                                hp[:],
                                wc1FT_sb[:, c * C2 + o * 128:
                                         c * C2 + (o + 1) * 128],
                                FT[c][:, sl],
                                start=(c == 0), stop=False)
                        nc.tensor.matmul(
                            hp[:], wq2T_sb[:, osl], e_sb[:, sl],
                            start=False, stop=True)
                        # copy to SBUF with +cst2 bias, accumulate sum(h)
                        nc.scalar.activation(
                            h_sb[o][:, sl], hp[:], ACT.Identity,
                            bias=cst2_sb[:, o:o + 1], accum_out=acc[b][:])
                        nc.scalar.activation(scr[:], h_sb[o][:, sl],
                                             ACT.Square, accum_out=sqa[b][:])
                    if nblk == 1:
                        nc.vector.tensor_copy(st1[:, o:o + 1], acc[0][:])
                        nc.vector.tensor_copy(st1[:, no2 + o:no2 + o + 1],
                                              sqa[0][:])
                    else:
                        nc.vector.tensor_add(st1[:, o:o + 1], acc[0][:],
                                             acc[1][:])
                        nc.vector.tensor_add(st1[:, no2 + o:no2 + o + 1],
                                             sqa[0][:], sqa[1][:])

            # sigma2 partial into st1 col 2*nо2; zero pad col
            nc.vector.reduce_sum(st1[:, 2 * no2:2 * no2 + 1], sg6[:], axis=AX.X)
            nc.vector.memset(st1[:, 2 * no2 + 1:2 * no2 + 2], 0.0)

            # ============== AR1: bn1 stats + sigma2 =========================
            with tc.tile_pool(name="arp", bufs=2, space="PSUM") as arp:
                st1tp = arp.tile([nst1, 128], f32, name="st1tp", tag="arp")
                nc.tensor.transpose(st1tp[:], st1[:], ident_sb[:])
                st1t = small.tile([nst1, 128], f32, name="st1t", bufs=1)
                nc.scalar.copy(st1t[:], st1tp[:])
                b1i = dram.tile([nst1, 128], f32, name="b1i")
                b1o = dram.tile([nst1, 128], f32, addr_space="Shared",
                                name="b1o")
                nc.sync.dma_start(b1i[:], st1t[:])
                nc.gpsimd.collective_compute(
                    "AllReduce", ALU.add, replica_groups=rg,
                    ins=[b1i.opt()], outs=[b1o.opt()])
                st1st = small.tile([nst1, 128], f32, name="st1st", bufs=1)
                nc.sync.dma_start(st1st[:], b1o[:])
                st1sp = arp.tile([128, nst1], f32, name="st1sp", tag="arp")
                nc.tensor.transpose(st1sp[:], st1st[:],
                                    ident_sb[0:nst1, 0:nst1])
                st1s = small.tile([128, nst1], f32, name="st1s", bufs=1)
                nc.scalar.copy(st1s[:], st1sp[:])

            # alpha/beta for bn1
            mh1 = small.tile([128, no2], f32, name="mh1", bufs=1)
            nc.vector.tensor_scalar_mul(mh1[:], st1s[:, 0:no2], inv_n)
            vh1 = small.tile([128, no2], f32, name="vh1", bufs=1)
            nc.vector.tensor_scalar_mul(vh1[:], st1s[:, no2:2 * no2], inv_n)
            msq = small.tile([128, no2], f32, name="msq", bufs=1)
            nc.vector.tensor_mul(msq[:], mh1[:], mh1[:])
            nc.vector.tensor_sub(vh1[:], vh1[:], msq[:])
            nc.vector.tensor_scalar_add(vh1[:], vh1[:], EPS)
            nc.scalar.sqrt(vh1[:], vh1[:])
            rsd = small.tile([128, no2], f32, name="rsd", bufs=1)
            nc.vector.reciprocal(rsd[:], vh1[:])
            al1 = small.tile([128, no2], f32, name="al1", bufs=1)
            nc.vector.tensor_mul(al1[:], bn1g_sb[:], rsd[:])
            be1 = small.tile([128, no2], f32, name="be1", bufs=1)
            nc.vector.tensor_mul(be1[:], al1[:], mh1[:])
            nc.vector.tensor_sub(be1[:], bn1b_sb[:], be1[:])

            # sigma2 total -> lse = LAMBDA*evals*sigma2
            sgp = small.tile([128, 1], f32, name="sgp", bufs=1)
            nc.gpsimd.partition_all_reduce(
                sgp[:], st1s[:, 2 * no2:2 * no2 + 1], channels=128,
                reduce_op=bass_isa.ReduceOp.add)
            lse = small.tile([128, 1], f32, name="lse", bufs=1)
            nc.vector.tensor_mul(lse[:], evl_sb[:], sgp[:])
            nc.vector.tensor_scalar_mul(lse[:], lse[:], inv_nc2)

            # ============== Phase 7: relu(bn1(h)); fp = F + wc2@. + cst3 ====
            rh = [pers.tile([128, ns], mmdt, name=f"rh{o}") for o in range(no2)]
            for o in range(no2):
                nc.scalar.activation(rh[o][:], h_sb[o][:], ACT.Relu,
                                     bias=be1[:, o:o + 1], scale=al1[:, o:o + 1])

            fp_sb = [pers.tile([128, ns], f32, name=f"fp{c}") for c in range(nf)]
            st2 = small.tile([128, nst2], f32, name="st2", bufs=1)
            with tc.tile_pool(name="fps", bufs=1, space="PSUM") as fpsp:
                for o in range(nf):
                    osl = slice(o * 128, (o + 1) * 128)
                    acc = [small.tile([128, 1], f32, name=f"facc{o}_{b}",
                                      tag="facc") for b in range(nblk)]
                    sqa = [small.tile([128, 1], f32, name=f"fsq{o}_{b}",
                                      tag="fsq") for b in range(nblk)]
                    for b in range(nblk):
                        sl = slice(b * blk, (b + 1) * blk)
                        fpp = fpsp.tile([128, blk], f32, name="fpp", tag="fpp",
                                        bufs=4)
                        for c in range(no2):
                            nc.tensor.matmul(
                                fpp[:],
                                wc2T_sb[:, c * C + o * 128:
                                        c * C + (o + 1) * 128],
                                rh[c][:, sl],
                                start=(c == 0), stop=(c == no2 - 1))
                        nc.vector.scalar_tensor_tensor(
                            fp_sb[o][:, sl], fpp[:], cst3_sb[:, o:o + 1],
                            FT[o][:, sl].bitcast(f32), op0=ALU.add, op1=ALU.add,
                            accum_out=acc[b][:])
                        nc.scalar.activation(scr[:], fp_sb[o][:, sl],
                                             ACT.Square, accum_out=sqa[b][:])
                    if nblk == 1:
                        nc.vector.tensor_copy(st2[:, o:o + 1], acc[0][:])
                        nc.vector.tensor_copy(st2[:, nf + o:nf + o + 1],
                                              sqa[0][:])
                    else:
                        nc.vector.tensor_add(st2[:, o:o + 1], acc[0][:],
                                             acc[1][:])
                        nc.vector.tensor_add(st2[:, nf + o:nf + o + 1],
                                             sqa[0][:], sqa[1][:])

            # ============== AR2: fw_bn stats ================================
            with tc.tile_pool(name="arp2", bufs=2, space="PSUM") as arp2:
                st2tp = arp2.tile([nst2, 128], f32, name="st2tp", tag="arp2")
                nc.tensor.transpose(st2tp[:], st2[:], ident_sb[:])
                st2t = small.tile([nst2, 128], f32, name="st2t", bufs=1)
                nc.scalar.copy(st2t[:], st2tp[:])
                b2i = dram.tile([nst2, 128], f32, name="b2i")
                b2o = dram.tile([nst2, 128], f32, addr_space="Shared",
                                name="b2o")
                nc.sync.dma_start(b2i[:], st2t[:])
                nc.gpsimd.collective_compute(
                    "AllReduce", ALU.add, replica_groups=rg,
                    ins=[b2i.opt()], outs=[b2o.opt()])
                st2st = small.tile([nst2, 128], f32, name="st2st", bufs=1)
                nc.sync.dma_start(st2st[:], b2o[:])
                st2sp = arp2.tile([128, nst2], f32, name="st2sp", tag="arp2")
                nc.tensor.transpose(st2sp[:], st2st[:],
                                    ident_sb[0:nst2, 0:nst2])
                st2s = small.tile([128, nst2], f32, name="st2s", bufs=1)
                nc.scalar.copy(st2s[:], st2sp[:])

            mh2 = small.tile([128, nf], f32, name="mh2", bufs=1)
            nc.vector.tensor_scalar_mul(mh2[:], st2s[:, 0:nf], inv_n)
            vh2 = small.tile([128, nf], f32, name="vh2", bufs=1)
            nc.vector.tensor_scalar_mul(vh2[:], st2s[:, nf:2 * nf], inv_n)
            msq2 = small.tile([128, nf], f32, name="msq2", bufs=1)
            nc.vector.tensor_mul(msq2[:], mh2[:], mh2[:])
            nc.vector.tensor_sub(vh2[:], vh2[:], msq2[:])
            nc.vector.tensor_scalar_add(vh2[:], vh2[:], EPS)
            nc.scalar.sqrt(vh2[:], vh2[:])
            rsd2 = small.tile([128, nf], f32, name="rsd2", bufs=1)
            nc.vector.reciprocal(rsd2[:], vh2[:])
            al2 = small.tile([128, nf], f32, name="al2", bufs=1)
            nc.vector.tensor_mul(al2[:], fwg_sb[:], rsd2[:])
            be2 = small.tile([128, nf], f32, name="be2", bufs=1)
            nc.vector.tensor_mul(be2[:], al2[:], mh2[:])
            nc.vector.tensor_sub(be2[:], fwbb_sb[:], be2[:])

            # ============== Phase 8: P = clip(sigmoid(fw conv)) =============
            with tc.tile_pool(name="psT2", bufs=1, space="PSUM") as psT2:
                rfp = [work.tile([128, ns], mmdt, name=f"rfp{c}", tag="rfp")
                       for c in range(nf)]
                for c in range(nf):
                    nc.scalar.activation(rfp[c][:], fp_sb[c][:], ACT.Relu,
                                         bias=be2[:, c:c + 1],
                                         scale=al2[:, c:c + 1])
                p_sb = small.tile([1, ns], f32, name="p_sb", bufs=1)
                for b in range(nblk):
                    sl = slice(b * blk, (b + 1) * blk)
                    zps = psT2.tile([1, blk], f32, name="zps", tag="zps", bufs=1)
                    for c in range(nf):
                        nc.tensor.matmul(
                            zps[:],
                            fwwT_sb[:, c:c + 1],
                            rfp[c][:, sl],
                            start=(c == 0), stop=(c == nf - 1))
                    nc.scalar.activation(p_sb[:, sl], zps[:], ACT.Sigmoid,
                                         bias=fwb0_sb[:, 0:1])
                nc.vector.tensor_scalar_min(p_sb[:], p_sb[:], MAX_V)
                nc.vector.tensor_scalar_max(p_sb[:], p_sb[:], MIN_V)
                nc.sync.dma_start(p_out.rearrange("(o n) -> o n", o=1), p_sb[:])

                # transpose p -> [128, nch]
                ptp = psT2.tile([128, nch], f32, name="ptp", tag="ptp", bufs=1)
                for i in range(nch):
                    nc.tensor.transpose(ptp[:, i:i + 1],
                                        p_sb[:, i * 128:(i + 1) * 128],
                                        ident_sb[0:1, 0:1])
                pt_sb = small.tile([128, nch], f32, name="pt_sb", bufs=1)
                nc.scalar.copy(pt_sb[:], ptp[:])

                # ========== Phase 9: Gram partial + AR3 =====================
                gram_sb = small.tile([128, K + 3], f32, name="gram_sb", bufs=1)
                with tc.tile_pool(name="grp", bufs=1, space="PSUM") as grp:
                    gram1 = grp.tile([128, K], f32, name="gram1")
                    gram2 = grp.tile([128, 4], f32, name="gram2")
                    pus = [work.tile([128, K], f32, name=f"pu{i}", tag="pu",
                                     bufs=3) for i in range(nch)]
                    for i in range(nch):
                        nc.vector.tensor_scalar_mul(
                            pus[i][:], usbig[:, i * K:(i + 1) * K],
                            pt_sb[:, i:i + 1])
                        nc.tensor.matmul(gram1[:], pus[i][:],
                                         usbig[:, i * K:(i + 1) * K],
                                         start=(i == 0), stop=(i == nch - 1))
                    for i in range(nch):
                        nc.tensor.matmul(gram2[:, 0:3], pus[i][:],
                                         vfb[:, i * 3:(i + 1) * 3],
                                         start=(i == 0), stop=(i == nch - 1))
                    nc.vector.tensor_copy(gram_sb[:, 0:K], gram1[:])
                    nc.vector.tensor_copy(gram_sb[:, K:K + 3], gram2[:, 0:3])

                b3i = dram.tile([128, K + 3], f32, name="b3i")
                b3o = dram.tile([128, K + 3], f32, addr_space="Shared",
                                name="b3o")
                nc.sync.dma_start(b3i[:], gram_sb[:])
                nc.gpsimd.collective_compute(
                    "AllReduce", ALU.add, replica_groups=rg,
                    ins=[b3i.opt()], outs=[b3o.opt()])
                g3s = small.tile([128, K + 3], f32, name="g3s", bufs=1)
                nc.sync.dma_start(g3s[:], b3o[:])

                # A = gram_sum + diag(lse)
                A_sb = small.tile([128, K], f32, name="A_sb", bufs=1)
                nc.vector.scalar_tensor_tensor(
                    A_sb[:], ident_sb[:], lse[:, 0:1], g3s[:, 0:K],
                    op0=ALU.mult, op1=ALU.add)

                # ========== Phase 10: Jacobi solve ==========================
                dg = small.tile([128, K], f32, name="dg", bufs=1)
                dvec = small.tile([128, 1], f32, name="dvec", bufs=1)
                nc.vector.tensor_mul(dg[:], A_sb[:], ident_sb[:])
                nc.vector.reduce_sum(dvec[:], dg[:], axis=AX.X)
                dinv = small.tile([128, 1], f32, name="dinv", bufs=1)
                nc.vector.reciprocal(dinv[:], dvec[:])
                with tc.tile_pool(name="jac", bufs=4) as jac:
                    x = jac.tile([128, 3], f32, name="x0", tag="x")
                    nc.vector.tensor_scalar_mul(x[:], g3s[:, K:K + 3],
                                                dinv[:, 0:1])
                    for it in range(NJACOBI):
                        axp = psT2.tile([128, 3], f32, name="axp", tag="axp",
                                        bufs=1)
                        nc.tensor.matmul(axp[:], A_sb[:], x[:],
                                         start=True, stop=True)
                        r = jac.tile([128, 3], f32, name="r", tag="x")
                        nc.vector.tensor_sub(r[:], g3s[:, K:K + 3], axp[:])
                        xn = jac.tile([128, 3], f32, name="xn", tag="x")
                        nc.vector.scalar_tensor_tensor(
                            xn[:], r[:], dinv[:, 0:1], x[:],
                            op0=ALU.mult, op1=ALU.add)
                        x = xn

                    # ====== Phase 11: T = U @ W + vert_x (output [3, ns]) ===
                    ttp = psT2.tile([3, ns], f32, name="ttp", tag="ttp", bufs=1)
                    for i in range(nch):
                        utp = psT2.tile([128, 128], f32, name="utp", tag="utp",
                                        bufs=1)
                        nc.tensor.transpose(utp[:], usbig[:, i * K:(i + 1) * K],
                                            ident_sb[:])
                        ut_sb = work.tile([128, 128], f32, name="ut_sb",
                                          tag="ut")
                        nc.scalar.copy(ut_sb[:], utp[:])
                        tps = psT2.tile([128, 3], f32, name="tps", tag="axp",
                                        bufs=1)
                        nc.tensor.matmul(tps[:], ut_sb[:], x[:],
                                         start=True, stop=True)
                        t_sb = work.tile([128, 3], f32, name="t_sb", tag="tsb")
                        nc.vector.tensor_add(
                            t_sb[:], tps[:], vxsbig[:, i * 3:(i + 1) * 3])
                        nc.tensor.transpose(ttp[:, i * 128:(i + 1) * 128],
                                            t_sb[:], ident_sb[:])
                    tt_sb = work.tile([3, ns], f32, name="tt_sb", tag="tsb2")
                    nc.scalar.copy(tt_sb[:], ttp[:])
                    nc.sync.dma_start(t_out[:], tt_sb[:])

    nc.compile()
    return nc


# --------------------------------------------------------------------------
# Host-side prep
# --------------------------------------------------------------------------

def host_prep(inputs, n_full=N_FULL, ncores=NCORES, deg=DEG, amax=AMAX):
    """Build per-core in_maps from full inputs (layout prep + tiny algebra)."""
    ns = n_full // ncores
    fp = np.float32
    g = {k: np.asarray(v, dtype=fp) for k, v in inputs.items()}

    s = g["loc_scores"].astype(np.float64)
    # Chebyshev fit of f(a) = sum(s e^{as})/sum(e^{as}) on [-amax, amax]
    M = 4 * (deg + 1)
    nodes = np.cos(np.pi * (np.arange(M) + 0.5) / M) * amax
    t = np.exp(nodes[:, None] * s[None, :])
    fe = (t * s).sum(1) / t.sum(1)
    ch = np.polynomial.chebyshev.Chebyshev.fit(nodes, fe, deg,
                                               domain=[-amax, amax])
    cc = ch.coef.astype(fp)                     # [deg+1]
    chb = np.repeat(cc[None, :], 128, axis=0)   # [128, deg+1]

    wk, wv, wq = g["wk"], g["wv"], g["wq"]
    w_up, b_up = g["w_up"][:, 0], g["b_up"]
    kv1 = wk @ w_up                              # [C]
    vv1 = wv @ w_up
    vv0 = wv @ b_up + g["bv"]
    sq = np.float64(np.sqrt(HD))
    waT = np.zeros((C, H), fp)
    bav = np.zeros((H, 1), fp)
    for h in range(H):
        sl = slice(h * HD, (h + 1) * HD)
        waT[:, h] = (kv1[sl] @ wq[sl, :]) / sq
        bav[h, 0] = (kv1[sl] @ g["bq"][sl]) / sq

    def chunk_rows(mat):
        # [R, X] -> [128, (R//128)*X]: col block k holds rows k*128..
        R, X = mat.shape
        return np.ascontiguousarray(
            mat.reshape(R // 128, 128, X).transpose(1, 0, 2).reshape(
                128, (R // 128) * X)).astype(fp)

    wmh, bmh = g["wmh"], g["bmh"]
    wv1 = wmh * vv1[None, :]                     # [C, C]
    wv1h = np.stack([wv1[:, h * HD:(h + 1) * HD].sum(1)
                     for h in range(H)], axis=1)  # [C, H]
    const_add = wmh @ vv0 + bmh                  # [C]
    wc1, bc1 = g["wc1"], g["bc1"]
    wc1F, wc1A = wc1[:, :C], wc1[:, C:]
    wq2 = wc1A @ wv1h                            # [C2, H]
    cst2v = wc1A @ const_add + bc1               # [C2]

    def chunk_cols(v):
        # [M] -> [128, M//128]  (channel c = col*128 + p)
        return np.ascontiguousarray(v.reshape(-1, 128).T).astype(fp)

    def pmajor(mat):
        # [nchunks*128, D] -> [128, nchunks*D], chunk k in col block k
        R, D = mat.shape
        return np.ascontiguousarray(
            mat.reshape(R // 128, 128, D).transpose(1, 0, 2).reshape(
                128, (R // 128) * D)).astype(fp)

    rep = {
        "fvr": pmajor(np.concatenate(
            [g["feat_y"][0], g["vert_y"][0],
             np.zeros((n_full, 1), fp)], axis=1)),
        "evl": (LAMBDA * g["evals_x"]).reshape(K, 1).astype(fp),
        "waT": chunk_rows(waT), "bav": bav, "chb": chb.astype(fp),
        "wc1FT": chunk_rows(np.ascontiguousarray(wc1F.T)),
        "wq2T": np.ascontiguousarray(wq2.T),
        "cst2": chunk_cols(cst2v),
        "bn1g": chunk_cols(g["bn1_g"]), "bn1b": chunk_cols(g["bn1_b"]),
        "wc2T": chunk_rows(np.ascontiguousarray(g["wc2"].T)),
        "cst3": chunk_cols(g["bc2"]),
        "fwg": chunk_cols(g["fw_bn_g"]), "fwbb": chunk_cols(g["fw_bn_b"]),
        "fwwT": chunk_rows(np.ascontiguousarray(g["fw_w"].T)),
        "fwb0": g["fw_b"].reshape(1, 1).astype(fp),
        "ident": np.eye(128, dtype=fp),
    }
    in_maps = []
    for c in range(ncores):
        sl = slice(c * ns, (c + 1) * ns)
        m = dict(rep)
        m["pxyT"] = np.ascontiguousarray(g["Pxy"][sl, :].T)
        m["fxsr"] = pmajor(g["feat_x"][0][sl, :])
        m["vxsr"] = pmajor(g["vert_x"][0][sl, :])
        m["usr"] = pmajor(g["evecs_x"][sl, :])
        in_maps.append(m)
    return in_maps


def assemble(results, n_full=N_FULL, ncores=NCORES):
    ns = n_full // ncores
    T = np.zeros((1, n_full, 3), np.float32)
    P = np.zeros((1, 1, n_full), np.float32)
    for c in range(ncores):
        sl = slice(c * ns, (c + 1) * ns)
        T[0, sl, :] = results[c]["t_out"].T
        P[0, 0, sl] = results[c]["p_out"]
    return T, P


def kernel(**inputs):
    key = "main"
    if key not in _NC_CACHE:
        _NC_CACHE[key] = build()
    nc = _NC_CACHE[key]
    in_maps = host_prep(inputs)
    res = bass_utils.run_bass_kernel_spmd(
        nc, in_maps, core_ids=list(range(NCORES)),
        trace=bool(os.environ.get("KERNEL_TRACE")))
    out = assemble(res.results)
    kernel.last_result = res
    return out


# revision 14
# speedup vs baseline: 1.2115x; 1.0066x over previous
"""Trainium2 Bass kernel for nn_DeformNet (8-core SPMD).

Strategy (N=6144 sharded across 8 cores, Ns=768 rows each):
  - The attention's K/V come from `losc = w_up (x) loc_scores + b_up`, which is
    rank-1 in the key index m.  Softmax logits per (head h, query n) are affine
    in s_m = loc_scores[m], so the full [H,N,M] attention collapses to
    E[h,n] = f(a[h,n]) with f(a) = sum_m s_m e^{a s_m} / sum_m e^{a s_m} and
    a = (kv1_h . q_h[:,n]) / sqrt(hd).  f is a smooth scalar function of a,
    evaluated on device via a Chebyshev/Clenshaw expansion whose coefficients
    are fitted on the host from the runtime loc_scores values (fp32-exact).
  - The multi-head mix + wc1 conv fold algebraically: h = wc1F @ F + wq2 @ E + cst2.
  - Per-core: G = Pxy_shard @ [feat_y | vert_y] on PE (fp32r), then the conv
    tail, BN stats via AllGather(+local sum), per-point weights P, partial
    Gram U^T diag(p) U and rhs, AllGather, replicated Jacobi solve (A is
    strongly diagonally dominant), T = U @ W + vert_x.
"""

import os
import sys
import types

import numpy as np


def _install_ntff_hook():
    """Make trace=True work under axon (antenv.axon_hooks is absent here)."""
    if "antenv.axon_hooks" in sys.modules:
        return
    try:
        import trn_agent_boot.trn_boot as tb
        hook = tb._ntff_profile_via_ctypes("/opt/axon/libaxon_pjrt.so")
        mod = types.ModuleType("antenv.axon_hooks")
        mod.get_axon_ntff_profile_hook = lambda: hook
        mod.set_axon_ntff_profile_hook = lambda h: None
        sys.modules["antenv.axon_hooks"] = mod
    except Exception:
        pass


_install_ntff_hook()

import concourse.bass as bass  # noqa: E402,F401
import concourse.bacc as bacc  # noqa: E402
import concourse.bass_isa as bass_isa  # noqa: E402
import concourse.tile as tile  # noqa: E402
import concourse.mybir as mybir  # noqa: E402
from concourse import bass_utils  # noqa: E402

f32 = mybir.dt.float32
f32r = mybir.dt.float32r
ALU = mybir.AluOpType
ACT = mybir.ActivationFunctionType
AX = mybir.AxisListType

# Problem constants
N_FULL, C, H, HD, K = 6144, 256, 4, 64, 128
NCORES = 8
LAMBDA, EPS, MIN_V, MAX_V = 10.0, 1e-5, 0.05, 0.95
AMAX, DEG = 16.0, 40
NJACOBI = 4
C2 = 2 * C

_NC_CACHE = {}


# --------------------------------------------------------------------------
# Device program
# --------------------------------------------------------------------------

def build(n_full=N_FULL, ncores=NCORES, use_f32r=True):
    """Build the SPMD program (identical on all cores; data differs)."""
    ns = n_full // ncores          # rows per core
    nch = ns // 128                # n-chunks (128 rows each)
    mch = n_full // 128            # m-chunks over the contraction dim
    nb = ns // 128                 # 128-wide blocks of the local free dim
    blk = 384 if ns % 384 == 0 else (256 if ns % 256 == 0 else 128)
    nblk = ns // blk               # wide free blocks for matmul streaming
    bpb = blk // 128               # 128-blocks per wide block
    mmdt = f32r if use_f32r else f32
    no2 = C2 // 128                # h channel chunks (4)
    nf = C // 128                  # feature channel chunks (2)

    nc = bacc.Bacc("TRN2", target_bir_lowering=False, debug=False,
                   num_devices=ncores, enable_asserts=False)

    def din(name, shape):
        return nc.dram_tensor(name, shape, f32, kind="ExternalInput").ap()

    gsz = 3 if mch % 3 == 0 else 2  # fv m-chunks per DMA group
    # per-core inputs (partition-major where it kills DMA descriptors)
    pxyT = din("pxyT", [n_full, ns])
    fxsr = din("fxsr", [128, nch * C])
    vxsr = din("vxsr", [128, nch * 3])
    usr = din("usr", [128, nch * K])
    # replicated inputs
    fvr = din("fvr", [128, mch * (C + 4)])  # feat_y|vert_y|pad, partition-major
    evl = din("evl", [K, 1])           # LAMBDA * evals
    waT = din("waT", [128, nf * H])    # (kv1_h^T wq_h)/sqrt(hd), chunked
    bav = din("bav", [H, 1])           # (kv1_h . bq_h)/sqrt(hd)
    chb = din("chb", [128, DEG + 1])   # Chebyshev coeffs, bcast over partitions
    wc1FT = din("wc1FT", [128, nf * C2])  # wc1[:, :C].T chunked
    wq2T = din("wq2T", [H, C2])        # (wc1[:, C:] @ wv1h).T
    cst2 = din("cst2", [128, no2])
    bn1g = din("bn1g", [128, no2])
    bn1b = din("bn1b", [128, no2])
    wc2T = din("wc2T", [128, no2 * C])
    cst3 = din("cst3", [128, nf])
    fwg = din("fwg", [128, nf])
    fwbb = din("fwbb", [128, nf])
    fwwT = din("fwwT", [128, nf])
    fwb0 = din("fwb0", [1, 1])
    ident = din("ident", [128, 128])

    t_out = nc.dram_tensor("t_out", [3, ns], f32, kind="ExternalOutput").ap()
    p_out = nc.dram_tensor("p_out", [ns], f32, kind="ExternalOutput").ap()

    rg = [list(range(ncores))]
    inv_n = 1.0 / float(n_full)
    inv_nc2 = 1.0 / (float(n_full) * float(C))
    nst1 = 2 * no2 + 2
    nst2 = 2 * nf

    with tile.TileContext(nc) as tc:
        with tc.tile_pool(name="const", bufs=1) as cpool, \
             tc.tile_pool(name="fvp", bufs=1) as fvpool, \
             tc.tile_pool(name="persist", bufs=1) as pers, \
             tc.tile_pool(name="work", bufs=3) as work, \
             tc.tile_pool(name="small", bufs=2) as small, \
             tc.tile_pool(name="pxy", bufs=10) as pxyp, \
             tc.tile_pool(name="dram", bufs=1, space="DRAM") as dram:

            # ---- constants to SBUF ----
            def cdma(name, src, shape, dt=f32):
                t = cpool.tile(shape, dt, name=name)
                nc.sync.dma_start(t[:], src[:].bitcast(dt))
                return t

            ident_sb = cdma("ident_sb", ident, [128, 128])
            chb_sb = cdma("chb_sb", chb, [128, DEG + 1])
            waT_sb = cdma("waT_sb", waT, [128, nf * H], dt=mmdt)
            bav_sb = cdma("bav_sb", bav, [H, 1])
            evl_sb = cdma("evl_sb", evl, [K, 1])
            wc1FT_sb = cdma("wc1FT_sb", wc1FT, [128, nf * C2], dt=mmdt)
            wq2T_sb = cdma("wq2T_sb", wq2T, [H, C2], dt=mmdt)
            cst2_sb = cdma("cst2_sb", cst2, [128, no2])
            bn1g_sb = cdma("bn1g_sb", bn1g, [128, no2])
            bn1b_sb = cdma("bn1b_sb", bn1b, [128, no2])
            wc2T_sb = cdma("wc2T_sb", wc2T, [128, no2 * C], dt=mmdt)
            cst3_sb = cdma("cst3_sb", cst3, [128, nf])
            fwg_sb = cdma("fwg_sb", fwg, [128, nf])
            fwbb_sb = cdma("fwbb_sb", fwbb, [128, nf])
            fwwT_sb = cdma("fwwT_sb", fwwT, [128, nf], dt=mmdt)
            fwb0_sb = cdma("fwb0_sb", fwb0, [1, 1])

            # ---- shard-local tensors (merged DMAs) ----
            fxsbig = pers.tile([128, nch * C], f32, name="fxsbig")
            nc.sync.dma_start(fxsbig[:], fxsr[:])
            vxsbig = pers.tile([128, nch * 3], f32, name="vxsbig")
            nc.sync.dma_start(vxsbig[:], vxsr[:])
            usbig = pers.tile([128, nch * K], f32, name="usbig")
            nc.sync.dma_start(usbig[:], usr[:])
            vfb = pers.tile([128, nch * 3], f32, name="vfb")

            # fv (feat_y | vert_y | pad): 8 grouped DMAs, m-chunk slices
            cw = C + 4
            ngr = mch // gsz
            fvg = [fvpool.tile([128, gsz * cw], mmdt, name=f"fvg{g}")
                   for g in range(ngr)]

            def fv_t(m):
                return fvg[m // gsz][:, (m % gsz) * cw:(m % gsz + 1) * cw]

            # warm up the collective firmware early with a tiny AllReduce
            warm = small.tile([1, 8], f32, name="warm", bufs=1)
            nc.vector.memset(warm[:], 0.0)
            wrmi = dram.tile([1, 8], f32, name="wrmi")
            wrmo = dram.tile([1, 8], f32, addr_space="Shared", name="wrmo")
            nc.sync.dma_start(wrmi[:], warm[:])
            nc.gpsimd.collective_compute(
                "AllReduce", ALU.add, replica_groups=rg,
                ins=[wrmi.opt()], outs=[wrmo.opt()])

            # ================= Phase 1: G = Pxy_sh @ [feat_y|vert_y] ========
            F_sb = [pers.tile([128, C], f32, name=f"F{i}") for i in range(nch)]
            sg6 = small.tile([128, nch], f32, name="sg6", bufs=1)
            with tc.tile_pool(name="gps", bufs=1, space="PSUM") as gpsp:
                gps = [gpsp.tile([128, C + 4], f32, name=f"gps{i}")
                       for i in range(nch)]
                for m in range(mch):
                    if m % gsz == 0:
                        g = m // gsz
                        nc.sync.dma_start(
                            fvg[g][:],
                            fvr[:, g * gsz * cw:(g + 1) * gsz * cw].bitcast(mmdt))
                    pxt = pxyp.tile([128, ns], mmdt, name="pxt")
                    nc.sync.dma_start(
                        pxt[:], pxyT[m * 128:(m + 1) * 128, :].bitcast(mmdt))
                    for i in range(nch):
                        nc.tensor.matmul(
                            gps[i][:],
                            pxt[:, i * 128:(i + 1) * 128],
                            fv_t(m),
                            start=(m == 0), stop=(m == mch - 1),
                        )

                # ====== Phase 2: F = G - fx ; vF = G - vx ; sigma2 ==========
                sq_scr = work.tile([128, C], f32, name="sq_scr", tag="scr")
                for i in range(nch):
                    nc.vector.tensor_sub(F_sb[i][:], gps[i][:, 0:C],
                                         fxsbig[:, i * C:(i + 1) * C])
                    nc.vector.tensor_sub(vfb[:, i * 3:(i + 1) * 3],
                                         gps[i][:, C:C + 3],
                                         vxsbig[:, i * 3:(i + 1) * 3])
                    nc.scalar.activation(sq_scr[:], F_sb[i][:], ACT.Square,
                                         accum_out=sg6[:, i:i + 1])

            # ============== Phase 3-5: FT, a, Clenshaw ======================
            FT = [pers.tile([128, ns], mmdt, name=f"FT{c}") for c in range(nf)]
            ef = small.tile([128, nb * H], f32, name="ef", bufs=1)
            e_sb = small.tile([H, ns], mmdt, name="e_sb", bufs=1)
            with tc.tile_pool(name="psT1", bufs=1, space="PSUM") as psT1:
                for i in range(nch):
                    for c in range(nf):
                        tp = psT1.tile([128, 128], f32, name="tp", tag="tp",
                                       bufs=2)
                        nc.tensor.transpose(tp[:],
                                            F_sb[i][:, c * 128:(c + 1) * 128],
                                            ident_sb[:])
                        nc.scalar.copy(FT[c][:, i * 128:(i + 1) * 128], tp[:])

                # a = waT^T @ FT + ba   [H, ns]
                a_sb = small.tile([H, ns], f32, name="a_sb", bufs=1)
                for b in range(nblk):
                    sl = slice(b * blk, (b + 1) * blk)
                    aps = psT1.tile([H, blk], f32, name="aps", tag="aps", bufs=2)
                    for c in range(nf):
                        nc.tensor.matmul(
                            aps[:],
                            waT_sb[:, c * H:(c + 1) * H],
                            FT[c][:, sl],
                            start=(c == 0), stop=(c == nf - 1))
                    nc.scalar.activation(a_sb[:, sl], aps[:], ACT.Identity,
                                         bias=bav_sb[:, 0:1])

                # transpose a[H, ns] -> af24 [128, nb*H] (col q = b*H + h)
                af24 = psT1.tile([128, nb * H], f32, name="af24", tag="af24",
                                 bufs=1)
                for b in range(nb):
                    nc.tensor.transpose(af24[:, b * H:(b + 1) * H],
                                        a_sb[:, b * 128:(b + 1) * 128],
                                        ident_sb[0:H, 0:H])
                # x = clip(a / AMAX, -1, 1);  t2 = 2x
                x_sb = small.tile([128, nb * H], f32, name="x_sb", bufs=1)
                nc.scalar.activation(x_sb[:], af24[:], ACT.Copy,
                                     scale=1.0 / AMAX)
                nc.vector.tensor_scalar_min(x_sb[:], x_sb[:], 1.0)
                nc.vector.tensor_scalar_max(x_sb[:], x_sb[:], -1.0)
                t2_sb = small.tile([128, nb * H], f32, name="t2_sb", bufs=1)
                nc.vector.tensor_scalar_mul(t2_sb[:], x_sb[:], 2.0)

                # Clenshaw recurrence for E = f(a)
                with tc.tile_pool(name="clen", bufs=4) as clp:
                    bprev = clp.tile([128, nb * H], f32, name="clp0", tag="cl")
                    nc.vector.memset(bprev[:], 0.0)
                    bcur = clp.tile([128, nb * H], f32, name="clc0", tag="cl")
                    nc.vector.memset(bcur[:], 0.0)
                    for kk in range(DEG, 0, -1):
                        tmp = clp.tile([128, nb * H], f32, name="clt", tag="cl")
                        nc.vector.tensor_mul(tmp[:], t2_sb[:], bcur[:])
                        bnew = clp.tile([128, nb * H], f32, name="cln", tag="cl")
                        nc.vector.scalar_tensor_tensor(
                            bnew[:], tmp[:], chb_sb[:, kk:kk + 1], bprev[:],
                            op0=ALU.add, op1=ALU.subtract)
                        bprev, bcur = bcur, bnew
                    # E_f = c0 + x*bcur - bprev
                    nc.vector.tensor_mul(ef[:], x_sb[:], bcur[:])
                    nc.vector.scalar_tensor_tensor(
                        ef[:], ef[:], chb_sb[:, 0:1], bprev[:],
                        op0=ALU.add, op1=ALU.subtract)

                # transpose back per block: E [H, ns]
                e4ps = psT1.tile([H, ns], f32, name="e4ps", tag="etp", bufs=1)
                for b in range(nb):
                    nc.tensor.transpose(e4ps[:, b * 128:(b + 1) * 128],
                                        ef[:, b * H:(b + 1) * H], ident_sb[:])
                nc.scalar.copy(e_sb[:], e4ps[:])

            # ============== Phase 6: h = wc1F@F + wq2@E + cst2; stats =======
            h_sb = [pers.tile([128, ns], f32, name=f"h{o}") for o in range(no2)]
            st1 = small.tile([128, nst1], f32, name="st1", bufs=1)
            scr = work.tile([128, blk], f32, name="hscr", tag="scr2")
            with tc.tile_pool(name="hps", bufs=1, space="PSUM") as hpsp:
                hps = {}
                for o in range(no2):
                    for b in range(nblk):
                        hps[o, b] = hpsp.tile([128, blk], f32,
                                              name=f"hp{o}_{b}")
                # F-contraction matmuls first (can run during Clenshaw)
                for o in range(no2):
                    osl = slice(o * 128, (o + 1) * 128)
                    for b in range(nblk):
                        sl = slice(b * blk, (b + 1) * blk)
                        for c in range(nf):
                            nc.tensor.matmul(
                                hps[o, b][:],
                                wc1FT_sb[:, c * C2 + o * 128:
                                         c * C2 + (o + 1) * 128],
                                FT[c][:, sl],
                                start=(c == 0), stop=False)
                # E-contraction finishers + copies + stats
                for o in range(no2):
                    osl = slice(o * 128, (o + 1) * 128)
                    acc = [small.tile([128, 1], f32, name=f"hacc{o}_{b}",
                                      tag="hacc") for b in range(nblk)]
                    sqa = [small.tile([128, 1], f32, name=f"hsq{o}_{b}",
                                      tag="hsq") for b in range(nblk)]
                    for b in range(nblk):
                        sl = slice(b * blk, (b + 1) * blk)
                        nc.tensor.matmul(
                            hps[o, b][:], wq2T_sb[:, osl], e_sb[:, sl],
                            start=False, stop=True)
                        nc.scalar.activation(
                            h_sb[o][:, sl], hps[o, b][:], ACT.Identity,
                            bias=cst2_sb[:, o:o + 1], accum_out=acc[b][:])
                        nc.scalar.activation(scr[:], h_sb[o][:, sl],
                                             ACT.Square, accum_out=sqa[b][:])
                    if nblk == 1:
                        nc.vector.tensor_copy(st1[:, o:o + 1], acc[0][:])
                        nc.vector.tensor_copy(st1[:, no2 + o:no2 + o + 1],
                                              sqa[0][:])
                    else:
                        nc.vector.tensor_add(st1[:, o:o + 1], acc[0][:],
                                             acc[1][:])
                        nc.vector.tensor_add(st1[:, no2 + o:no2 + o + 1],
                                             sqa[0][:], sqa[1][:])

            # sigma2 partial into st1 col 2*nо2; zero pad col
            nc.vector.reduce_sum(st1[:, 2 * no2:2 * no2 + 1], sg6[:], axis=AX.X)
            nc.vector.memset(st1[:, 2 * no2 + 1:2 * no2 + 2], 0.0)

            # ============== AR1: bn1 stats + sigma2 =========================
            with tc.tile_pool(name="arp", bufs=2, space="PSUM") as arp:
                st1tp = arp.tile([nst1, 128], f32, name="st1tp", tag="arp")
                nc.tensor.transpose(st1tp[:], st1[:], ident_sb[:])
                st1t = small.tile([nst1, 128], f32, name="st1t", bufs=1)
                nc.scalar.copy(st1t[:], st1tp[:])
                b1i = dram.tile([nst1, 128], f32, name="b1i")
                b1o = dram.tile([nst1, 128], f32, addr_space="Shared",
                                name="b1o")
                nc.sync.dma_start(b1i[:], st1t[:])
                nc.gpsimd.collective_compute(
                    "AllReduce", ALU.add, replica_groups=rg,
                    ins=[b1i.opt()], outs=[b1o.opt()])
                st1st = small.tile([nst1, 128], f32, name="st1st", bufs=1)
                nc.sync.dma_start(st1st[:], b1o[:])
                st1sp = arp.tile([128, nst1], f32, name="st1sp", tag="arp")
                nc.tensor.transpose(st1sp[:], st1st[:],
                                    ident_sb[0:nst1, 0:nst1])
                st1s = small.tile([128, nst1], f32, name="st1s", bufs=1)
                nc.scalar.copy(st1s[:], st1sp[:])

            # alpha/beta for bn1
            mh1 = small.tile([128, no2], f32, name="mh1", bufs=1)
            nc.vector.tensor_scalar_mul(mh1[:], st1s[:, 0:no2], inv_n)
            vh1 = small.tile([128, no2], f32, name="vh1", bufs=1)
            nc.vector.tensor_scalar_mul(vh1[:], st1s[:, no2:2 * no2], inv_n)
            msq = small.tile([128, no2], f32, name="msq", bufs=1)
            nc.vector.tensor_mul(msq[:], mh1[:], mh1[:])
            nc.vector.tensor_sub(vh1[:], vh1[:], msq[:])
            nc.vector.tensor_scalar_add(vh1[:], vh1[:], EPS)
            nc.scalar.sqrt(vh1[:], vh1[:])
            rsd = small.tile([128, no2], f32, name="rsd", bufs=1)
            nc.vector.reciprocal(rsd[:], vh1[:])
            al1 = small.tile([128, no2], f32, name="al1", bufs=1)
            nc.vector.tensor_mul(al1[:], bn1g_sb[:], rsd[:])
            be1 = small.tile([128, no2], f32, name="be1", bufs=1)
            nc.vector.tensor_mul(be1[:], al1[:], mh1[:])
            nc.vector.tensor_sub(be1[:], bn1b_sb[:], be1[:])

            # sigma2 total -> lse = LAMBDA*evals*sigma2
            sgp = small.tile([128, 1], f32, name="sgp", bufs=1)
            nc.gpsimd.partition_all_reduce(
                sgp[:], st1s[:, 2 * no2:2 * no2 + 1], channels=128,
                reduce_op=bass_isa.ReduceOp.add)
            lse = small.tile([128, 1], f32, name="lse", bufs=1)
            nc.vector.tensor_mul(lse[:], evl_sb[:], sgp[:])
            nc.vector.tensor_scalar_mul(lse[:], lse[:], inv_nc2)

            # ============== Phase 7: relu(bn1(h)); fp = F + wc2@. + cst3 ====
            rh = [pers.tile([128, ns], mmdt, name=f"rh{o}") for o in range(no2)]
            for o in range(no2):
                nc.scalar.activation(rh[o][:], h_sb[o][:], ACT.Relu,
                                     bias=be1[:, o:o + 1], scale=al1[:, o:o + 1])

            fp_sb = [pers.tile([128, ns], f32, name=f"fp{c}") for c in range(nf)]
            st2 = small.tile([128, nst2], f32, name="st2", bufs=1)
            with tc.tile_pool(name="fps", bufs=1, space="PSUM") as fpsp:
                for o in range(nf):
                    osl = slice(o * 128, (o + 1) * 128)
                    acc = [small.tile([128, 1], f32, name=f"facc{o}_{b}",
                                      tag="facc") for b in range(nblk)]
                    sqa = [small.tile([128, 1], f32, name=f"fsq{o}_{b}",
                                      tag="fsq") for b in range(nblk)]
                    for b in range(nblk):
                        sl = slice(b * blk, (b + 1) * blk)
                        fpp = fpsp.tile([128, blk], f32, name="fpp", tag="fpp",
                                        bufs=4)
                        for c in range(no2):
                            nc.tensor.matmul(
                                fpp[:],
                                wc2T_sb[:, c * C + o * 128:
                                        c * C + (o + 1) * 128],
                                rh[c][:, sl],
                                start=(c == 0), stop=(c == no2 - 1))
                        nc.vector.scalar_tensor_tensor(
                            fp_sb[o][:, sl], fpp[:], cst3_sb[:, o:o + 1],
                            FT[o][:, sl].bitcast(f32), op0=ALU.add, op1=ALU.add,
                            accum_out=acc[b][:])
                        nc.scalar.activation(scr[:], fp_sb[o][:, sl],
                                             ACT.Square, accum_out=sqa[b][:])
                    if nblk == 1:
                        nc.vector.tensor_copy(st2[:, o:o + 1], acc[0][:])
                        nc.vector.tensor_copy(st2[:, nf + o:nf + o + 1],
                                              sqa[0][:])
                    else:
                        nc.vector.tensor_add(st2[:, o:o + 1], acc[0][:],
                                             acc[1][:])
                        nc.vector.tensor_add(st2[:, nf + o:nf + o + 1],
                                             sqa[0][:], sqa[1][:])

            # ============== AR2: fw_bn stats ================================
            with tc.tile_pool(name="arp2", bufs=2, space="PSUM") as arp2:
                st2tp = arp2.tile([nst2, 128], f32, name="st2tp", tag="arp2")
                nc.tensor.transpose(st2tp[:], st2[:], ident_sb[:])
                st2t = small.tile([nst2, 128], f32, name="st2t", bufs=1)
                nc.scalar.copy(st2t[:], st2tp[:])
                b2i = dram.tile([nst2, 128], f32, name="b2i")
                b2o = dram.tile([nst2, 128], f32, addr_space="Shared",
                                name="b2o")
                nc.sync.dma_start(b2i[:], st2t[:])
                nc.gpsimd.collective_compute(
                    "AllReduce", ALU.add, replica_groups=rg,
                    ins=[b2i.opt()], outs=[b2o.opt()])
                st2st = small.tile([nst2, 128], f32, name="st2st", bufs=1)
                nc.sync.dma_start(st2st[:], b2o[:])
                st2sp = arp2.tile([128, nst2], f32, name="st2sp", tag="arp2")
                nc.tensor.transpose(st2sp[:], st2st[:],
                                    ident_sb[0:nst2, 0:nst2])
                st2s = small.tile([128, nst2], f32, name="st2s", bufs=1)
                nc.scalar.copy(st2s[:], st2sp[:])

            mh2 = small.tile([128, nf], f32, name="mh2", bufs=1)
            nc.vector.tensor_scalar_mul(mh2[:], st2s[:, 0:nf], inv_n)
            vh2 = small.tile([128, nf], f32, name="vh2", bufs=1)
            nc.vector.tensor_scalar_mul(vh2[:], st2s[:, nf:2 * nf], inv_n)
            msq2 = small.tile([128, nf], f32, name="msq2", bufs=1)
            nc.vector.tensor_mul(msq2[:], mh2[:], mh2[:])
            nc.vector.tensor_sub(vh2[:], vh2[:], msq2[:])
            nc.vector.tensor_scalar_add(vh2[:], vh2[:], EPS)
            nc.scalar.sqrt(vh2[:], vh2[:])
            rsd2 = small.tile([128, nf], f32, name="rsd2", bufs=1)
            nc.vector.reciprocal(rsd2[:], vh2[:])
            al2 = small.tile([128, nf], f32, name="al2", bufs=1)
            nc.vector.tensor_mul(al2[:], fwg_sb[:], rsd2[:])
            be2 = small.tile([128, nf], f32, name="be2", bufs=1)
            nc.vector.tensor_mul(be2[:], al2[:], mh2[:])
            nc.vector.tensor_sub(be2[:], fwbb_sb[:], be2[:])

            # ============== Phase 8: P = clip(sigmoid(fw conv)) =============
            with tc.tile_pool(name="psT2", bufs=1, space="PSUM") as psT2:
                rfp = [work.tile([128, ns], mmdt, name=f"rfp{c}", tag="rfp")
                       for c in range(nf)]
                for c in range(nf):
                    nc.scalar.activation(rfp[c][:], fp_sb[c][:], ACT.Relu,
                                         bias=be2[:, c:c + 1],
                                         scale=al2[:, c:c + 1])
                p_sb = small.tile([1, ns], f32, name="p_sb", bufs=1)
                for b in range(nblk):
                    sl = slice(b * blk, (b + 1) * blk)
                    zps = psT2.tile([1, blk], f32, name="zps", tag="zps", bufs=1)
                    for c in range(nf):
                        nc.tensor.matmul(
                            zps[:],
                            fwwT_sb[:, c:c + 1],
                            rfp[c][:, sl],
                            start=(c == 0), stop=(c == nf - 1))
                    nc.scalar.activation(p_sb[:, sl], zps[:], ACT.Sigmoid,
                                         bias=fwb0_sb[:, 0:1])
                nc.vector.tensor_scalar_min(p_sb[:], p_sb[:], MAX_V)
                nc.vector.tensor_scalar_max(p_sb[:], p_sb[:], MIN_V)
                nc.sync.dma_start(p_out.rearrange("(o n) -> o n", o=1), p_sb[:])

                # transpose p -> [128, nch]
                ptp = psT2.tile([128, nch], f32, name="ptp", tag="ptp", bufs=1)
                for i in range(nch):
                    nc.tensor.transpose(ptp[:, i:i + 1],
                                        p_sb[:, i * 128:(i + 1) * 128],
                                        ident_sb[0:1, 0:1])
                pt_sb = small.tile([128, nch], f32, name="pt_sb", bufs=1)
                nc.scalar.copy(pt_sb[:], ptp[:])

                # ========== Phase 9: Gram partial + AR3 =====================
                gram_sb = small.tile([128, K + 3], f32, name="gram_sb", bufs=1)
                with tc.tile_pool(name="grp", bufs=1, space="PSUM") as grp:
                    gram1 = grp.tile([128, K], f32, name="gram1")
                    gram2 = grp.tile([128, 4], f32, name="gram2")
                    pus = [work.tile([128, K], f32, name=f"pu{i}", tag="pu",
                                     bufs=3) for i in range(nch)]
                    for i in range(nch):
                        nc.vector.tensor_scalar_mul(
                            pus[i][:], usbig[:, i * K:(i + 1) * K],
                            pt_sb[:, i:i + 1])
                        nc.tensor.matmul(gram1[:], pus[i][:],
                                         usbig[:, i * K:(i + 1) * K],
                                         start=(i == 0), stop=(i == nch - 1))
                    for i in range(nch):
                        nc.tensor.matmul(gram2[:, 0:3], pus[i][:],
                                         vfb[:, i * 3:(i + 1) * 3],
                                         start=(i == 0), stop=(i == nch - 1))
                    nc.vector.tensor_copy(gram_sb[:, 0:K], gram1[:])
                    nc.vector.tensor_copy(gram_sb[:, K:K + 3], gram2[:, 0:3])

                b3i = dram.tile([128, K + 3], f32, name="b3i")
                b3o = dram.tile([128, K + 3], f32, addr_space="Shared",
                                name="b3o")
                for q in range(4):
                    nc.sync.dma_start(b3i[q * 32:(q + 1) * 32, :],
                                      gram_sb[q * 32:(q + 1) * 32, :])
                nc.gpsimd.collective_compute(
                    "AllReduce", ALU.add, replica_groups=rg,
                    ins=[b3i.opt()], outs=[b3o.opt()])
                g3s = small.tile([128, K + 3], f32, name="g3s", bufs=1)
                for q in range(4):
                    nc.sync.dma_start(g3s[q * 32:(q + 1) * 32, :],
                                      b3o[q * 32:(q + 1) * 32, :])

                # A = gram_sum + diag(lse)
                A_sb = small.tile([128, K], f32, name="A_sb", bufs=1)
                nc.vector.scalar_tensor_tensor(
                    A_sb[:], ident_sb[:], lse[:, 0:1], g3s[:, 0:K],
                    op0=ALU.mult, op1=ALU.add)

                # ========== Phase 10: Jacobi solve ==========================
                dg = small.tile([128, K], f32, name="dg", bufs=1)
                dvec = small.tile([128, 1], f32, name="dvec", bufs=1)
                nc.vector.tensor_mul(dg[:], A_sb[:], ident_sb[:])
                nc.vector.reduce_sum(dvec[:], dg[:], axis=AX.X)
                dinv = small.tile([128, 1], f32, name="dinv", bufs=1)
                nc.vector.reciprocal(dinv[:], dvec[:])
                with tc.tile_pool(name="jac", bufs=4) as jac:
                    x = jac.tile([128, 3], f32, name="x0", tag="x")
                    nc.vector.tensor_scalar_mul(x[:], g3s[:, K:K + 3],
                                                dinv[:, 0:1])
                    for it in range(NJACOBI):
                        axp = psT2.tile([128, 3], f32, name="axp", tag="axp",
                                        bufs=1)
                        nc.tensor.matmul(axp[:], A_sb[:], x[:],
                                         start=True, stop=True)
                        r = jac.tile([128, 3], f32, name="r", tag="x")
                        nc.vector.tensor_sub(r[:], g3s[:, K:K + 3], axp[:])
                        xn = jac.tile([128, 3], f32, name="xn", tag="x")
                        nc.vector.scalar_tensor_tensor(
                            xn[:], r[:], dinv[:, 0:1], x[:],
                            op0=ALU.mult, op1=ALU.add)
                        x = xn

                    # ====== Phase 11: T = U @ W + vert_x (output [3, ns]) ===
                    ttp = psT2.tile([3, ns], f32, name="ttp", tag="ttp", bufs=1)
                    for i in range(nch):
                        utp = psT2.tile([128, 128], f32, name="utp", tag="utp",
                                        bufs=1)
                        nc.tensor.transpose(utp[:], usbig[:, i * K:(i + 1) * K],
                                            ident_sb[:])
                        ut_sb = work.tile([128, 128], f32, name="ut_sb",
                                          tag="ut")
                        nc.scalar.copy(ut_sb[:], utp[:])
                        tps = psT2.tile([128, 3], f32, name="tps", tag="axp",
                                        bufs=1)
                        nc.tensor.matmul(tps[:], ut_sb[:], x[:],
                                         start=True, stop=True)
                        t_sb = work.tile([128, 3], f32, name="t_sb", tag="tsb")
                        nc.vector.tensor_add(
                            t_sb[:], tps[:], vxsbig[:, i * 3:(i + 1) * 3])
                        nc.tensor.transpose(ttp[:, i * 128:(i + 1) * 128],
                                            t_sb[:], ident_sb[:])
                    tt_sb = work.tile([3, ns], f32, name="tt_sb", tag="tsb2")
                    nc.scalar.copy(tt_sb[:], ttp[:])
                    nc.sync.dma_start(t_out[:], tt_sb[:])

    nc.compile()
    return nc


# --------------------------------------------------------------------------
# Host-side prep
# --------------------------------------------------------------------------

def host_prep(inputs, n_full=N_FULL, ncores=NCORES, deg=DEG, amax=AMAX):
    """Build per-core in_maps from full inputs (layout prep + tiny algebra)."""
    ns = n_full // ncores
    fp = np.float32
    g = {k: np.asarray(v, dtype=fp) for k, v in inputs.items()}

    s = g["loc_scores"].astype(np.float64)
    # Chebyshev fit of f(a) = sum(s e^{as})/sum(e^{as}) on [-amax, amax]
    M = 4 * (deg + 1)
    nodes = np.cos(np.pi * (np.arange(M) + 0.5) / M) * amax
    t = np.exp(nodes[:, None] * s[None, :])
    fe = (t * s).sum(1) / t.sum(1)
    ch = np.polynomial.chebyshev.Chebyshev.fit(nodes, fe, deg,
                                               domain=[-amax, amax])
    cc = ch.coef.astype(fp)                     # [deg+1]
    chb = np.repeat(cc[None, :], 128, axis=0)   # [128, deg+1]

    wk, wv, wq = g["wk"], g["wv"], g["wq"]
    w_up, b_up = g["w_up"][:, 0], g["b_up"]
    kv1 = wk @ w_up                              # [C]
    vv1 = wv @ w_up
    vv0 = wv @ b_up + g["bv"]
    sq = np.float64(np.sqrt(HD))
    waT = np.zeros((C, H), fp)
    bav = np.zeros((H, 1), fp)
    for h in range(H):
        sl = slice(h * HD, (h + 1) * HD)
        waT[:, h] = (kv1[sl] @ wq[sl, :]) / sq
        bav[h, 0] = (kv1[sl] @ g["bq"][sl]) / sq

    def chunk_rows(mat):
        # [R, X] -> [128, (R//128)*X]: col block k holds rows k*128..
        R, X = mat.shape
        return np.ascontiguousarray(
            mat.reshape(R // 128, 128, X).transpose(1, 0, 2).reshape(
                128, (R // 128) * X)).astype(fp)

    wmh, bmh = g["wmh"], g["bmh"]
    wv1 = wmh * vv1[None, :]                     # [C, C]
    wv1h = np.stack([wv1[:, h * HD:(h + 1) * HD].sum(1)
                     for h in range(H)], axis=1)  # [C, H]
    const_add = wmh @ vv0 + bmh                  # [C]
    wc1, bc1 = g["wc1"], g["bc1"]
    wc1F, wc1A = wc1[:, :C], wc1[:, C:]
    wq2 = wc1A @ wv1h                            # [C2, H]
    cst2v = wc1A @ const_add + bc1               # [C2]

    def chunk_cols(v):
        # [M] -> [128, M//128]  (channel c = col*128 + p)
        return np.ascontiguousarray(v.reshape(-1, 128).T).astype(fp)

    def pmajor(mat):
        # [nchunks*128, D] -> [128, nchunks*D], chunk k in col block k
        R, D = mat.shape
        return np.ascontiguousarray(
            mat.reshape(R // 128, 128, D).transpose(1, 0, 2).reshape(
                128, (R // 128) * D)).astype(fp)

    rep = {
        "fvr": pmajor(np.concatenate(
            [g["feat_y"][0], g["vert_y"][0],
             np.zeros((n_full, 1), fp)], axis=1)),
        "evl": (LAMBDA * g["evals_x"]).reshape(K, 1).astype(fp),
        "waT": chunk_rows(waT), "bav": bav, "chb": chb.astype(fp),
        "wc1FT": chunk_rows(np.ascontiguousarray(wc1F.T)),
        "wq2T": np.ascontiguousarray(wq2.T),
        "cst2": chunk_cols(cst2v),
        "bn1g": chunk_cols(g["bn1_g"]), "bn1b": chunk_cols(g["bn1_b"]),
        "wc2T": chunk_rows(np.ascontiguousarray(g["wc2"].T)),
        "cst3": chunk_cols(g["bc2"]),
        "fwg": chunk_cols(g["fw_bn_g"]), "fwbb": chunk_cols(g["fw_bn_b"]),
        "fwwT": chunk_rows(np.ascontiguousarray(g["fw_w"].T)),
        "fwb0": g["fw_b"].reshape(1, 1).astype(fp),
        "ident": np.eye(128, dtype=fp),
    }
    in_maps = []
    for c in range(ncores):
        sl = slice(c * ns, (c + 1) * ns)
        m = dict(rep)
        m["pxyT"] = np.ascontiguousarray(g["Pxy"][sl, :].T)
        m["fxsr"] = pmajor(g["feat_x"][0][sl, :])
        m["vxsr"] = pmajor(g["vert_x"][0][sl, :])
        m["usr"] = pmajor(g["evecs_x"][sl, :])
        in_maps.append(m)
    return in_maps


def assemble(results, n_full=N_FULL, ncores=NCORES):
    ns = n_full // ncores
    T = np.zeros((1, n_full, 3), np.float32)
    P = np.zeros((1, 1, n_full), np.float32)
    for c in range(ncores):
        sl = slice(c * ns, (c + 1) * ns)
        T[0, sl, :] = results[c]["t_out"].T
        P[0, 0, sl] = results[c]["p_out"]
    return T, P


def kernel(**inputs):
    key = "main"
    if key not in _NC_CACHE:
        _NC_CACHE[key] = build()
    nc = _NC_CACHE[key]
    in_maps = host_prep(inputs)
    res = bass_utils.run_bass_kernel_spmd(
        nc, in_maps, core_ids=list(range(NCORES)),
        trace=bool(os.environ.get("KERNEL_TRACE")))
    out = assemble(res.results)
    kernel.last_result = res
    return out
